# revision 20
# baseline (speedup 1.0000x reference)
"""Trainium2 Bass kernel for the quantum-circuit KG-embedding scoring model.

Math: score(s,p,o) = Re(<B_o h | W_p | B_s h>) where B_e / W_p are the
24-gate circuit blocks for entity/relation params and h = |+>^6.

v4 design (fp16, interleaved re/im pairs):
  State layout [128 part, nt, 128] fp16 where the 128 free elems are 64
  amplitudes as interleaved (re, im) pairs.  A 2x2 gate update is 14
  tensor_tensor ops (8 pair-coeff products + 6 adds; re/im swaps are
  negative-stride reads of contiguous temps), all DVE "2x_1p" eligible.

  Phase A: evolve 1280 local entities (10 tiles); store fp16 rows,
    AllGather -> T_full [10240, 128].
  Phase W: evolve 16 W tiles (2 relations x 64 basis cols each) on DVE
    WHILE phase C's gathers run on the Pool engine; W^T slots assembled
    into SBUF wsb [128, 32, 128] via 4 partition-strided SBUF->SBUF DMAs
    (even rows = basis states, odd rows = swap-negate).
  Phase C: host packs the p-sorted batch into a STATIC slot->tile map
    (slot s = t//3; every relation on a core needs <= 3 tiles, <= 32
    relations per core - verified for B=65536, R=200).  Per tile: two
    single-index indirect row-gathers (s, o), XBAR DMA transpose of Ts,
    PE matmul Y^T = Ts @ W^T_slot (static SBUF rhs), DVE product with
    To, ACT-engine accumulate into scores.

Host does only: trig for the 200-relation coeff tables (tiny), index
sort/packing, and output unpermute.
"""

import sys
import numpy as np

for _p in ("/opt/trn_rl_repo",):
    if _p not in sys.path:
        sys.path.insert(0, _p)

import concourse.bass as bass
import concourse.bacc as bacc
import concourse.mybir as mybir
from concourse import tile
from concourse.bass_utils import run_bass_kernel_spmd

F32 = mybir.dt.float32
F16 = mybir.dt.float16
I32 = mybir.dt.int32
ALU = mybir.AluOpType
ACTFN = mybir.ActivationFunctionType

P = 128
Q = 6
NA = 64                      # 2^Q amplitudes
NCORES = 8
E, R, B = 10000, 200, 65536
ETILES = 10                  # entity tiles per core
EPC = ETILES * P             # 1280 entities per core
EPAD = EPC * NCORES          # 10240 padded entity rows
WTILES = 16                  # W-phase tiles per core (2 relations each)
RSLOT = 2 * WTILES           # 32 relation slots per core
TPS = 3                      # static tiles per slot
NT = RSLOT * TPS             # 96 phase-C tiles per core
R2 = float(2.0 ** -0.5)
PI = float(np.pi)

# CRot gate list: (control, target) wire pairs, in circuit order
CROTS = [(q, (q + off) % Q) for off in (1, 2, 3) for q in range(Q)]


# --------------------------------------------------------------------------
# device program
# --------------------------------------------------------------------------


def _pair_bc(coef_ap, nt, nrep):
    """[p, nt, 2] pair slice -> broadcast to [p, nt, nrep, 2] (4-D)."""
    v = coef_ap.rearrange("p n (m two) -> p n m two", m=1, two=2)
    return v.to_broadcast([P, nt, nrep, 2])


def _emit_crot(nc, pool, st, ec, nt, g, c, t, tag):
    """Apply CRot gate g (control c, target t) in place on st [P, nt, 128].

    ec: [P, nt, 24, 8] fp16 pair-coeff slots (v0,v0, -v1,v1, v2,v2, -v3,v3).
    """
    cpos, tpos = 5 - c, 5 - t
    hi, lo = max(cpos, tpos), min(cpos, tpos)
    A = 1 << (5 - hi)
    Bm = 1 << (hi - lo - 1)
    C = 1 << lo
    v = st.rearrange(
        "p n (a x b y c two) -> p n a x b y c two",
        a=A, x=2, b=Bm, y=2, c=C, two=2,
    )
    cbit_is_x = cpos == hi

    def sel(cv, tv):
        xv, yv = (cv, tv) if cbit_is_x else (tv, cv)
        return v[:, :, :, xv, :, yv, :, :]

    a0 = sel(1, 0)   # [p, n, A, Bm, C, 2]
    a1 = sel(1, 1)
    L = A * Bm * C * 2  # 32

    def co(lo_s):
        return _pair_bc(ec[:, :, g, lo_s : lo_s + 2], nt, A * Bm * C)

    def qt(tg):
        tt = pool.tile([P, nt, L], F16, tag=tag + tg)
        return tt

    def as5(tt):
        return tt[:].rearrange(
            "p n (a b c two) -> p n a b c two", a=A, b=Bm, c=C, two=2
        )

    def flat(tt):
        return tt[:].rearrange("p n (l two) -> p n l two", two=2)

    def swap(tt):
        return flat(tt)[:, :, :, ::-1]

    c0, c1, c2, c3 = co(0), co(2), co(4), co(6)
    q1, q2, q3, q4 = qt("q1"), qt("q2"), qt("q3"), qt("q4")
    q5, q6, q7, q8 = qt("q5"), qt("q6"), qt("q7"), qt("q8")
    TT = nc.vector.tensor_tensor
    TT(out=as5(q1), in0=a0, in1=c0, op=ALU.mult)
    TT(out=as5(q2), in0=a0, in1=c1, op=ALU.mult)
    TT(out=as5(q3), in0=a1, in1=c2, op=ALU.mult)
    TT(out=as5(q4), in0=a1, in1=c3, op=ALU.mult)
    TT(out=as5(q5), in0=a0, in1=c2, op=ALU.mult)
    TT(out=as5(q6), in0=a0, in1=c3, op=ALU.mult)
    TT(out=as5(q7), in0=a1, in1=c0, op=ALU.mult)
    TT(out=as5(q8), in0=a1, in1=c1, op=ALU.mult)
    pa, pb = qt("pa"), qt("pb")
    pc, pd = qt("pc"), qt("pd")
    TT(out=flat(pa), in0=flat(q1), in1=swap(q2), op=ALU.add)
    TT(out=flat(pb), in0=flat(q3), in1=swap(q4), op=ALU.subtract)
    TT(out=flat(pc), in0=flat(q5), in1=swap(q6), op=ALU.add)
    TT(out=flat(pd), in0=flat(q7), in1=swap(q8), op=ALU.subtract)
    TT(out=a0, in0=as5(pa), in1=as5(pb), op=ALU.subtract)
    TT(out=a1, in0=as5(pc), in1=as5(pd), op=ALU.add)


def _pair_bc_db(coef_ap, nt, w):
    v = coef_ap.rearrange("p n (w two) -> p n w two", w=1, two=2)
    return v.to_broadcast([P, nt, w, 2])


def _emit_doubling(nc, pool, st, fac, nt, tag):
    """Product-state doubling, in place on st [P, nt, 128] fp16.

    fac [P, nt, 6, 8] fp16 slots per step: (f0r,f0r, -f0i,f0i, f1r,f1r,
    -f1i,f1i); step k expands amplitude bit k (wire 5-k).
    """
    CP = nc.vector.tensor_copy
    TT = nc.vector.tensor_tensor
    # seed from step-0 factors: amp0 = f0, amp1 = f1
    CP(out=st[:, :, 0:1], in_=fac[:, :, 0, 0:1])
    CP(out=st[:, :, 1:2], in_=fac[:, :, 0, 3:4])
    CP(out=st[:, :, 2:3], in_=fac[:, :, 0, 4:5])
    CP(out=st[:, :, 3:4], in_=fac[:, :, 0, 7:8])
    for k in range(1, 6):
        w = 1 << k  # current state width in pairs
        cview = st[:, :, 0 : 2 * w].rearrange("p n (w two) -> p n w two", two=2)
        for m in (1, 0):  # m=1 writes fresh upper half first
            frp = _pair_bc_db(fac[:, :, k, 4 * m : 4 * m + 2], nt, w)
            fim = _pair_bc_db(fac[:, :, k, 4 * m + 2 : 4 * m + 4], nt, w)
            t1 = pool.tile([P, nt, 2 * w], F16, tag=tag + "A")
            t2 = pool.tile([P, nt, 2 * w], F16, tag=tag + "B")
            t1v = t1[:].rearrange("p n (w two) -> p n w two", two=2)
            t2v = t2[:].rearrange("p n (w two) -> p n w two", two=2)
            TT(out=t1v, in0=cview, in1=frp, op=ALU.mult)
            TT(out=t2v, in0=cview, in1=fim, op=ALU.mult)
            t1f = t1[:].rearrange("p n (w two) -> p n w two", two=2)
            t2s = t2[:].rearrange("p n (w two) -> p n w two", two=2)[:, :, :, ::-1]
            df = st[:, :, m * 2 * w : (m + 1) * 2 * w].rearrange(
                "p n (w two) -> p n w two", two=2
            )
            TT(out=df, in0=t1f, in1=t2s, op=ALU.subtract)


def build_program(no_collective=False):
    nc = bacc.Bacc("TRN2", target_bir_lowering=False, debug=False)

    ent = nc.dram_tensor("ent_par", [ETILES, P, 72], F32, kind="ExternalInput")
    wcoef_d = nc.dram_tensor("wcoef", [P, WTILES, 24, 8], F16, kind="ExternalInput")
    wfac_d = nc.dram_tensor("wfac", [P, WTILES, 6, 8], F16, kind="ExternalInput")
    sidx_d = nc.dram_tensor("sidx", [P, NT], I32, kind="ExternalInput")
    oidx_d = nc.dram_tensor("oidx", [P, NT], I32, kind="ExternalInput")
    scores_d = nc.dram_tensor("scores", [P, NT], F32, kind="ExternalOutput")

    with tile.TileContext(nc) as tc:
        with (
            tc.tile_pool(name="const", bufs=1) as cp,
            tc.tile_pool(name="gtmp", bufs=2) as gp,
            tc.tile_pool(name="gts", bufs=6) as gtsp,
            tc.tile_pool(name="gto", bufs=6) as gtop,
            tc.tile_pool(name="tst", bufs=6) as tstp,
            tc.tile_pool(name="scr", bufs=4) as scrp,
            tc.tile_pool(name="cpy", bufs=4, space="PSUM") as psY,
            tc.tile_pool(name="dram", bufs=1, space="DRAM") as dp,
        ):
            # ---------------- DRAM scratch ----------------
            T_loc = dp.tile([EPC, P], F16)
            T_full = dp.tile([EPAD, P], F16, addr_space="Shared")
            W_loc = dp.tile([WTILES, 2, NA, 2, P], F16)

            # ---------------- load inputs ----------------
            ang = cp.tile([P, ETILES, 72], F32)
            nc.sync.dma_start(out=ang[:], in_=ent[:].rearrange("t p k -> p t k"))
            ec = cp.tile([P, ETILES + WTILES, 24, 8], F16)
            fac = cp.tile([P, ETILES + WTILES, 6, 8], F16)
            nc.sync.dma_start(out=ec[:, ETILES:], in_=wcoef_d[:])
            nc.sync.dma_start(out=fac[:, ETILES:], in_=wfac_d[:])
            sidx = cp.tile([P, NT], I32)
            nc.sync.dma_start(out=sidx[:], in_=sidx_d[:])
            oidx = cp.tile([P, NT], I32)
            nc.sync.dma_start(out=oidx[:], in_=oidx_d[:])

            pm = cp.tile([P, 2], F16)
            nc.vector.memset(pm[:, 0:1], -1.0)
            nc.vector.memset(pm[:, 1:2], 1.0)

            cdb = cp.tile([P, 3], F32)
            nc.vector.memset(cdb[:, 0:1], 0.0)
            nc.vector.memset(cdb[:, 1:2], 0.5)
            nc.vector.memset(cdb[:, 2:3], PI / 2)
            nc.const_aps.aps[(F32, 0.0)] = cdb[:, 0:1]
            nc.const_aps.aps[(F32, 0.5)] = cdb[:, 1:2]
            nc.const_aps.aps[(F32, PI / 2)] = cdb[:, 2:3]

            # ---------------- entity coeffs (A part) ----------------
            av = ang[:].rearrange("p t (g a) -> p t g a", g=24, a=3)
            phi, tha, omg = av[:, :, :, 0], av[:, :, :, 1], av[:, :, :, 2]
            s1 = cp.tile([P, ETILES, 24], F32)
            s2 = cp.tile([P, ETILES, 24], F32)
            nc.vector.tensor_tensor(out=s1[:], in0=phi, in1=omg, op=ALU.add)
            nc.vector.tensor_tensor(out=s2[:], in0=phi, in1=omg, op=ALU.subtract)

            half = cp.tile([P, ETILES, 6, 24], F32)
            trig = cp.tile([P, ETILES, 6, 24], F32)
            hv, tv = half[:], trig[:]
            for i, srcv in ((0, tha), (2, s1[:]), (4, s2[:])):
                nc.vector.tensor_scalar(
                    out=hv[:, :, i], in0=srcv, scalar1=0.5, scalar2=PI / 2,
                    op0=ALU.mult, op1=ALU.add,
                )
                nc.vector.tensor_scalar_mul(hv[:, :, i + 1], srcv, 0.5)
            for i in range(6):
                nc.scalar.activation(out=tv[:, :, i], in_=hv[:, :, i], func=ACTFN.Sin)

            # products -> pair slots (v0,v0, -v1,v1, v2,v2, -v3,v3), fp16
            eA = ec[:, 0:ETILES]
            ch, sh = tv[:, :, 0], tv[:, :, 1]
            ca, sa = tv[:, :, 2], tv[:, :, 3]
            cb, sb = tv[:, :, 4], tv[:, :, 5]

            def u1(x):
                return x.rearrange("p t (g two) -> p t g two", two=1)

            def b2(x):
                return u1(x).to_broadcast([P, ETILES, 24, 2])

            TT = nc.vector.tensor_tensor
            TS = nc.vector.tensor_scalar_mul
            TT(out=eA[:, :, :, 0:2], in0=b2(ch), in1=b2(ca), op=ALU.mult)
            TT(out=eA[:, :, :, 3:4], in0=u1(ch), in1=u1(sa), op=ALU.mult)
            TS(eA[:, :, :, 2:3], eA[:, :, :, 3:4], -1.0)
            TT(out=eA[:, :, :, 4:6], in0=b2(sh), in1=b2(cb), op=ALU.mult)
            TT(out=eA[:, :, :, 7:8], in0=u1(sh), in1=u1(sb), op=ALU.mult)
            TS(eA[:, :, :, 6:7], eA[:, :, :, 7:8], -1.0)

            # layer-0 |+> doubling factors; step k expands wire 5-k (gate 5-k)
            fA = fac[:, 0:ETILES]
            rev = eA[:, :, 5::-1, :]
            v0r, v1r = rev[:, :, :, 0:1], rev[:, :, :, 3:4]
            v2r, v3r = rev[:, :, :, 4:5], rev[:, :, :, 7:8]
            tmp6 = cp.tile([P, ETILES, 6, 1], F32)
            t6 = tmp6[:]

            def fpair(dst_lo, n, scale):
                src = t6.to_broadcast([P, ETILES, 6, n]) if n == 2 else t6
                TS(fA[:, :, :, dst_lo : dst_lo + n], src, scale)

            TT(out=t6, in0=v0r, in1=v2r, op=ALU.subtract)
            fpair(0, 2, R2)                       # (f0r, f0r)
            TT(out=t6, in0=v1r, in1=v3r, op=ALU.add)
            fpair(2, 1, R2)                       # -f0i  (f0i = -(v1+v3)*r2)
            fpair(3, 1, -R2)                      # +f0i
            TT(out=t6, in0=v0r, in1=v2r, op=ALU.add)
            fpair(4, 2, R2)                       # (f1r, f1r)
            TT(out=t6, in0=v1r, in1=v3r, op=ALU.subtract)
            fpair(6, 1, -R2)                      # -f1i  (f1i = (v1-v3)*r2)
            fpair(7, 1, R2)                       # +f1i

            # ---------------- phase A evolution ----------------
            stA = cp.tile([P, ETILES, P], F16)
            _emit_doubling(nc, gp, stA[:], fac[:, 0:ETILES], ETILES, "adb")
            for g, (c, t) in enumerate(CROTS):
                _emit_crot(nc, gp, stA[:], ec[:, 0:ETILES], ETILES, 6 + g, c, t, "ac")

            # T store + AllGather
            nc.sync.dma_start(
                out=T_loc[:].rearrange("(t p) k -> p t k", p=P), in_=stA[:]
            )
            if no_collective:
                nc.sync.dma_start(out=T_full[0:EPC, :], in_=T_loc[:])
            else:
                nc.gpsimd.collective_compute(
                    "AllGather",
                    ALU.bypass,
                    ins=[T_loc[:]],
                    outs=[T_full[:]],
                    replica_groups=[list(range(NCORES))],
                )

            # ---------------- phase W evolution ----------------
            stW = cp.tile([P, WTILES, P], F16)
            _emit_doubling(nc, gp, stW[:], fac[:, ETILES:], WTILES, "wdb")
            for g, (c, t) in enumerate(CROTS):
                _emit_crot(nc, gp, stW[:], ec[:, ETILES:], WTILES, 6 + g, c, t, "wc")

            # W^T slots into SBUF: even rows = basis states, odd = swap-negate
            tempw = cp.tile([P, WTILES, P], F16)
            sw_in = stW[:].rearrange("p n (w two) -> p (n w) two", two=2)[:, :, ::-1]
            pmb = pm[:].rearrange("p (o two) -> p o two", o=1).to_broadcast(
                [P, WTILES * NA, 2]
            )
            nc.vector.tensor_tensor(
                out=tempw[:].rearrange("p n (w two) -> p (n w) two", two=2),
                in0=sw_in, in1=pmb, op=ALU.mult,
            )
            wl = W_loc[:].rearrange("wt h j two k -> (h j) wt two k")
            nc.sync.dma_start(out=wl[:, :, 0, :], in_=stW[:])
            nc.sync.dma_start(out=wl[:, :, 1, :], in_=tempw[:])
            wsb = cp.tile([P, RSLOT, P], F16)
            nc.sync.dma_start(
                out=wsb[:],
                in_=W_loc[:].rearrange("wt h j two k -> (j two) (wt h) k"),
            )

            # ---------------- phase C ----------------
            scores = cp.tile([P, NT], F32)
            T_flat = T_full[:]
            for t in range(NT):
                gts = gtsp.tile([P, P], F16, tag="gts")
                nc.gpsimd.indirect_dma_start(
                    out=gts[:], out_offset=None, in_=T_flat,
                    in_offset=bass.IndirectOffsetOnAxis(
                        ap=sidx[:, t : t + 1], axis=0),
                )
                gto = gtop.tile([P, P], F16, tag="gto")
                nc.gpsimd.indirect_dma_start(
                    out=gto[:], out_offset=None, in_=T_flat,
                    in_offset=bass.IndirectOffsetOnAxis(
                        ap=oidx[:, t : t + 1], axis=0),
                )
                tst = tstp.tile([P, P], F16, tag="tst")
                eng = nc.sync if (t % 2 == 0) else nc.scalar
                eng.dma_start(out=tst[:], in_=gts[:], transpose=True)
                pY = psY.tile([P, P], F32, tag="py")
                nc.tensor.matmul(
                    out=pY[:], lhsT=tst[:], rhs=wsb[:, t // TPS, :],
                    start=True, stop=True,
                )
                scr = scrp.tile([P, P], F32, tag="scr")
                nc.vector.tensor_tensor(
                    out=scr[:], in0=gto[:], in1=pY[:], op=ALU.mult
                )
                sdum = scrp.tile([P, P], F32, tag="sdum")
                nc.scalar.activation(
                    out=sdum[:], in_=scr[:], func=ACTFN.Copy,
                    accum_out=scores[:, t : t + 1],
                )

            nc.sync.dma_start(out=scores_d[:], in_=scores[:])

    nc.finalize()
    return nc


# --------------------------------------------------------------------------
# host side
# --------------------------------------------------------------------------


def _rot_elems(params):
    """params [..., 3] (phi, theta, omega) -> v0, v1, v2, v3 arrays.

    m00=(v0,-v1) m01=(-v2,-v3) m10=(v2,-v3) m11=(v0,v1)
    """
    phi, tha, omg = params[..., 0], params[..., 1], params[..., 2]
    ch, sh = np.cos(tha / 2), np.sin(tha / 2)
    a, b = (phi + omg) / 2, (phi - omg) / 2
    return ch * np.cos(a), ch * np.sin(a), sh * np.cos(b), sh * np.sin(b)


def _host_prep(entity_params, relation_params, s_idx, p_idx, o_idx):
    ent = np.asarray(entity_params, dtype=np.float32)
    rel = np.asarray(relation_params, dtype=np.float32)
    s_idx = np.asarray(s_idx)
    p_idx = np.asarray(p_idx)
    o_idx = np.asarray(o_idx)

    # ---- entity shards ----
    ent_flat = ent.reshape(E, 72)
    ent_pad = np.zeros((EPAD, 72), np.float32)
    ent_pad[:E] = ent_flat
    ent_shards = [
        ent_pad[c * EPC : (c + 1) * EPC].reshape(ETILES, P, 72) for c in range(NCORES)
    ]

    # ---- p-sorted stream, contiguous core slices ----
    order = np.argsort(p_idx, kind="stable")
    per = B // NCORES
    gate_rel = rel.reshape(R, 24, 3)
    v0, v1, v2, v3 = _rot_elems(gate_rel)  # each [R, 24]

    in_maps = []
    outpos = np.full((NCORES, NT, P), -1, np.int64)
    for c in range(NCORES):
        sl = order[c * per : (c + 1) * per]
        rels_c = p_idx[sl]
        rels, starts = np.unique(rels_c, return_index=True)
        assert len(rels) <= RSLOT, f"core {c} has {len(rels)} relations"

        sidx = np.zeros((P, NT), np.int32)
        oidx = np.zeros((P, NT), np.int32)
        for s, r in enumerate(rels):
            elems = sl[rels_c == r]
            assert len(elems) <= TPS * P, f"relation {r} needs >3 tiles"
            for k in range(TPS):
                seg = elems[k * P : (k + 1) * P]
                n = len(seg)
                if n == 0:
                    break
                t = s * TPS + k
                sidx[:n, t] = s_idx[seg]
                oidx[:n, t] = o_idx[seg]
                outpos[c, t, :n] = seg

        # W coeff pair table [P, WTILES, 24, 8]
        wcoef = np.zeros((P, WTILES, 24, 8), np.float16)
        wfac = np.zeros((P, WTILES, 6, 8), np.float16)
        j = np.arange(NA)
        for s, r in enumerate(rels):
            wt, hhalf = divmod(s, 2)
            rows = slice(NA * hhalf, NA * hhalf + NA)
            for g in range(24):
                vals = (v0[r, g], v0[r, g], -v1[r, g], v1[r, g],
                        v2[r, g], v2[r, g], -v3[r, g], v3[r, g])
                for kk, vv in enumerate(vals):
                    wcoef[rows, wt, g, kk] = vv
            # basis doubling factors: step k expands wire 5-k; basis j bit
            # of wire q=5-k is (j >> k) & 1
            for k in range(6):
                qg = 5 - k
                bit = (j >> k) & 1
                m00 = (v0[r, qg], -v1[r, qg])
                m01 = (-v2[r, qg], -v3[r, qg])
                m10 = (v2[r, qg], -v3[r, qg])
                m11 = (v0[r, qg], v1[r, qg])
                wfac[rows, wt, k, 0] = np.where(bit == 0, m00[0], m01[0])
                wfac[rows, wt, k, 1] = wfac[rows, wt, k, 0]
                u0i = np.where(bit == 0, m00[1], m01[1])
                wfac[rows, wt, k, 2] = -u0i
                wfac[rows, wt, k, 3] = u0i
                wfac[rows, wt, k, 4] = np.where(bit == 0, m10[0], m11[0])
                wfac[rows, wt, k, 5] = wfac[rows, wt, k, 4]
                u1i = np.where(bit == 0, m10[1], m11[1])
                wfac[rows, wt, k, 6] = -u1i
                wfac[rows, wt, k, 7] = u1i

        in_maps.append(
            {
                "ent_par": ent_shards[c],
                "wcoef": wcoef,
                "wfac": wfac,
                "sidx": sidx,
                "oidx": oidx,
            }
        )
    return in_maps, outpos


_PROGRAM = None


def kernel(entity_params, relation_params, s_idx, p_idx, o_idx):
    global _PROGRAM
    in_maps, outpos = _host_prep(entity_params, relation_params, s_idx, p_idx, o_idx)
    if _PROGRAM is None:
        _PROGRAM = build_program()
    nc = _PROGRAM
    res = run_bass_kernel_spmd(nc, in_maps, list(range(NCORES)))
    out = np.zeros(B, np.float32)
    for c in range(NCORES):
        sc = res.results[c]["scores"]  # [P, NT]
        pos = outpos[c]  # [NT, P]
        mask = pos >= 0
        out[pos[mask]] = sc.T[mask]
    return out


if __name__ == "__main__":
    build_program()
    print("build OK")


# revision 21
# speedup vs baseline: 1.0210x; 1.0210x over previous
"""Trainium2 Bass kernel for the quantum-circuit KG-embedding scoring model.

Math: score(s,p,o) = Re(<B_o h | W_p | B_s h>) where B_e / W_p are the
24-gate circuit blocks for entity/relation params and h = |+>^6.

v4 design (fp16, interleaved re/im pairs):
  State layout [128 part, nt, 128] fp16 where the 128 free elems are 64
  amplitudes as interleaved (re, im) pairs.  A 2x2 gate update is 14
  tensor_tensor ops (8 pair-coeff products + 6 adds; re/im swaps are
  negative-stride reads of contiguous temps), all DVE "2x_1p" eligible.

  Phase A: evolve 1280 local entities (10 tiles); store fp16 rows,
    AllGather -> T_full [10240, 128].
  Phase W: evolve 16 W tiles (2 relations x 64 basis cols each) on DVE
    WHILE phase C's gathers run on the Pool engine; W^T slots assembled
    into SBUF wsb [128, 32, 128] via 4 partition-strided SBUF->SBUF DMAs
    (even rows = basis states, odd rows = swap-negate).
  Phase C: host packs the p-sorted batch into a STATIC slot->tile map
    (slot s = t//3; every relation on a core needs <= 3 tiles, <= 32
    relations per core - verified for B=65536, R=200).  Per tile: two
    single-index indirect row-gathers (s, o), XBAR DMA transpose of Ts,
    PE matmul Y^T = Ts @ W^T_slot (static SBUF rhs), DVE product with
    To, ACT-engine accumulate into scores.

Host does only: trig for the 200-relation coeff tables (tiny), index
sort/packing, and output unpermute.
"""

import sys
import numpy as np

for _p in ("/opt/trn_rl_repo",):
    if _p not in sys.path:
        sys.path.insert(0, _p)

import concourse.bass as bass
import concourse.bacc as bacc
import concourse.mybir as mybir
from concourse import tile
from concourse.bass_utils import run_bass_kernel_spmd

F32 = mybir.dt.float32
F16 = mybir.dt.float16
I32 = mybir.dt.int32
ALU = mybir.AluOpType
ACTFN = mybir.ActivationFunctionType

P = 128
Q = 6
NA = 64                      # 2^Q amplitudes
NCORES = 8
E, R, B = 10000, 200, 65536
ETILES = 10                  # entity tiles per core
EPC = ETILES * P             # 1280 entities per core
EPAD = EPC * NCORES          # 10240 padded entity rows
WTILES = 16                  # W-phase tiles per core (2 relations each)
RSLOT = 2 * WTILES           # 32 relation slots per core
TPS = 3                      # static tiles per slot
NT = RSLOT * TPS             # 96 phase-C tiles per core
R2 = float(2.0 ** -0.5)
PI = float(np.pi)

# CRot gate list: (control, target) wire pairs, in circuit order
CROTS = [(q, (q + off) % Q) for off in (1, 2, 3) for q in range(Q)]


# --------------------------------------------------------------------------
# device program
# --------------------------------------------------------------------------


def _pair_bc(coef_ap, nt, nrep):
    """[p, nt, 2] pair slice -> broadcast to [p, nt, nrep, 2] (4-D)."""
    v = coef_ap.rearrange("p n (m two) -> p n m two", m=1, two=2)
    return v.to_broadcast([P, nt, nrep, 2])


def _emit_crot(nc, pool, st, ec, nt, g, c, t, tag):
    """Apply CRot gate g (control c, target t) in place on st [P, nt, 128].

    ec: [P, nt, 24, 8] fp16 pair-coeff slots (v0,v0, -v1,v1, v2,v2, -v3,v3).
    """
    cpos, tpos = 5 - c, 5 - t
    hi, lo = max(cpos, tpos), min(cpos, tpos)
    A = 1 << (5 - hi)
    Bm = 1 << (hi - lo - 1)
    C = 1 << lo
    v = st.rearrange(
        "p n (a x b y c two) -> p n a x b y c two",
        a=A, x=2, b=Bm, y=2, c=C, two=2,
    )
    cbit_is_x = cpos == hi

    def sel(cv, tv):
        xv, yv = (cv, tv) if cbit_is_x else (tv, cv)
        return v[:, :, :, xv, :, yv, :, :]

    a0 = sel(1, 0)   # [p, n, A, Bm, C, 2]
    a1 = sel(1, 1)
    L = A * Bm * C * 2  # 32

    def co(lo_s):
        return _pair_bc(ec[:, :, g, lo_s : lo_s + 2], nt, A * Bm * C)

    def qt(tg):
        tt = pool.tile([P, nt, L], F16, tag=tag + tg)
        return tt

    def as5(tt):
        return tt[:].rearrange(
            "p n (a b c two) -> p n a b c two", a=A, b=Bm, c=C, two=2
        )

    def flat(tt):
        return tt[:].rearrange("p n (l two) -> p n l two", two=2)

    def swap(tt):
        return flat(tt)[:, :, :, ::-1]

    c0, c1, c2, c3 = co(0), co(2), co(4), co(6)
    q1, q2, q3, q4 = qt("q1"), qt("q2"), qt("q3"), qt("q4")
    q5, q6, q7, q8 = qt("q5"), qt("q6"), qt("q7"), qt("q8")
    TT = nc.vector.tensor_tensor
    TT(out=as5(q1), in0=a0, in1=c0, op=ALU.mult)
    TT(out=as5(q2), in0=a0, in1=c1, op=ALU.mult)
    TT(out=as5(q3), in0=a1, in1=c2, op=ALU.mult)
    TT(out=as5(q4), in0=a1, in1=c3, op=ALU.mult)
    TT(out=as5(q5), in0=a0, in1=c2, op=ALU.mult)
    TT(out=as5(q6), in0=a0, in1=c3, op=ALU.mult)
    TT(out=as5(q7), in0=a1, in1=c0, op=ALU.mult)
    TT(out=as5(q8), in0=a1, in1=c1, op=ALU.mult)
    pa, pb = qt("pa"), qt("pb")
    pc, pd = qt("pc"), qt("pd")
    TT(out=flat(pa), in0=flat(q1), in1=swap(q2), op=ALU.add)
    TT(out=flat(pb), in0=flat(q3), in1=swap(q4), op=ALU.subtract)
    TT(out=flat(pc), in0=flat(q5), in1=swap(q6), op=ALU.add)
    TT(out=flat(pd), in0=flat(q7), in1=swap(q8), op=ALU.subtract)
    TT(out=a0, in0=as5(pa), in1=as5(pb), op=ALU.subtract)
    TT(out=a1, in0=as5(pc), in1=as5(pd), op=ALU.add)


def _pair_bc_db(coef_ap, nt, w):
    v = coef_ap.rearrange("p n (w two) -> p n w two", w=1, two=2)
    return v.to_broadcast([P, nt, w, 2])


def _emit_doubling(nc, pool, st, fac, nt, tag):
    """Product-state doubling, in place on st [P, nt, 128] fp16.

    fac [P, nt, 6, 8] fp16 slots per step: (f0r,f0r, -f0i,f0i, f1r,f1r,
    -f1i,f1i); step k expands amplitude bit k (wire 5-k).
    """
    CP = nc.vector.tensor_copy
    TT = nc.vector.tensor_tensor
    # seed from step-0 factors: amp0 = f0, amp1 = f1
    CP(out=st[:, :, 0:1], in_=fac[:, :, 0, 0:1])
    CP(out=st[:, :, 1:2], in_=fac[:, :, 0, 3:4])
    CP(out=st[:, :, 2:3], in_=fac[:, :, 0, 4:5])
    CP(out=st[:, :, 3:4], in_=fac[:, :, 0, 7:8])
    for k in range(1, 6):
        w = 1 << k  # current state width in pairs
        cview = st[:, :, 0 : 2 * w].rearrange("p n (w two) -> p n w two", two=2)
        for m in (1, 0):  # m=1 writes fresh upper half first
            frp = _pair_bc_db(fac[:, :, k, 4 * m : 4 * m + 2], nt, w)
            fim = _pair_bc_db(fac[:, :, k, 4 * m + 2 : 4 * m + 4], nt, w)
            t1 = pool.tile([P, nt, 2 * w], F16, tag=tag + "A")
            t2 = pool.tile([P, nt, 2 * w], F16, tag=tag + "B")
            t1v = t1[:].rearrange("p n (w two) -> p n w two", two=2)
            t2v = t2[:].rearrange("p n (w two) -> p n w two", two=2)
            TT(out=t1v, in0=cview, in1=frp, op=ALU.mult)
            TT(out=t2v, in0=cview, in1=fim, op=ALU.mult)
            t1f = t1[:].rearrange("p n (w two) -> p n w two", two=2)
            t2s = t2[:].rearrange("p n (w two) -> p n w two", two=2)[:, :, :, ::-1]
            df = st[:, :, m * 2 * w : (m + 1) * 2 * w].rearrange(
                "p n (w two) -> p n w two", two=2
            )
            TT(out=df, in0=t1f, in1=t2s, op=ALU.subtract)


def build_program(no_collective=False):
    nc = bacc.Bacc("TRN2", target_bir_lowering=False, debug=False)

    ent = nc.dram_tensor("ent_par", [ETILES, P, 72], F32, kind="ExternalInput")
    wcoef_d = nc.dram_tensor("wcoef", [P, WTILES, 24, 8], F16, kind="ExternalInput")
    wfac_d = nc.dram_tensor("wfac", [P, WTILES, 6, 8], F16, kind="ExternalInput")
    sidx_d = nc.dram_tensor("sidx", [P, NT], I32, kind="ExternalInput")
    oidx_d = nc.dram_tensor("oidx", [P, NT], I32, kind="ExternalInput")
    ident_d = nc.dram_tensor("ident", [P, P], F16, kind="ExternalInput")
    scores_d = nc.dram_tensor("scores", [P, NT], F32, kind="ExternalOutput")

    with tile.TileContext(nc) as tc:
        with (
            tc.tile_pool(name="const", bufs=1) as cp,
            tc.tile_pool(name="gtmp", bufs=2) as gp,
            tc.tile_pool(name="gts", bufs=6) as gtsp,
            tc.tile_pool(name="gto", bufs=6) as gtop,
            tc.tile_pool(name="tst", bufs=6) as tstp,
            tc.tile_pool(name="scr", bufs=4) as scrp,
            tc.tile_pool(name="cpy", bufs=4, space="PSUM") as psY,
            tc.tile_pool(name="dram", bufs=1, space="DRAM") as dp,
        ):
            # ---------------- DRAM scratch ----------------
            T_loc = dp.tile([EPC, P], F16)
            T_full = dp.tile([EPAD, P], F16, addr_space="Shared")
            W_loc = dp.tile([WTILES, 2, NA, 2, P], F16)

            # ---------------- load inputs ----------------
            ang = cp.tile([P, ETILES, 72], F32)
            nc.sync.dma_start(out=ang[:], in_=ent[:].rearrange("t p k -> p t k"))
            ec = cp.tile([P, ETILES + WTILES, 24, 8], F16)
            fac = cp.tile([P, ETILES + WTILES, 6, 8], F16)
            nc.sync.dma_start(out=ec[:, ETILES:], in_=wcoef_d[:])
            nc.sync.dma_start(out=fac[:, ETILES:], in_=wfac_d[:])
            sidx = cp.tile([P, NT], I32)
            nc.sync.dma_start(out=sidx[:], in_=sidx_d[:])
            oidx = cp.tile([P, NT], I32)
            nc.sync.dma_start(out=oidx[:], in_=oidx_d[:])

            pm = cp.tile([P, 2], F16)
            nc.vector.memset(pm[:, 0:1], -1.0)
            nc.vector.memset(pm[:, 1:2], 1.0)

            cdb = cp.tile([P, 3], F32)
            nc.vector.memset(cdb[:, 0:1], 0.0)
            nc.vector.memset(cdb[:, 1:2], 0.5)
            nc.vector.memset(cdb[:, 2:3], PI / 2)
            nc.const_aps.aps[(F32, 0.0)] = cdb[:, 0:1]
            nc.const_aps.aps[(F32, 0.5)] = cdb[:, 1:2]
            nc.const_aps.aps[(F32, PI / 2)] = cdb[:, 2:3]

            # ---------------- entity coeffs (A part) ----------------
            av = ang[:].rearrange("p t (g a) -> p t g a", g=24, a=3)
            phi, tha, omg = av[:, :, :, 0], av[:, :, :, 1], av[:, :, :, 2]
            s1 = cp.tile([P, ETILES, 24], F32)
            s2 = cp.tile([P, ETILES, 24], F32)
            nc.vector.tensor_tensor(out=s1[:], in0=phi, in1=omg, op=ALU.add)
            nc.vector.tensor_tensor(out=s2[:], in0=phi, in1=omg, op=ALU.subtract)

            half = cp.tile([P, ETILES, 6, 24], F32)
            trig = cp.tile([P, ETILES, 6, 24], F32)
            hv, tv = half[:], trig[:]
            for i, srcv in ((0, tha), (2, s1[:]), (4, s2[:])):
                nc.vector.tensor_scalar(
                    out=hv[:, :, i], in0=srcv, scalar1=0.5, scalar2=PI / 2,
                    op0=ALU.mult, op1=ALU.add,
                )
                nc.vector.tensor_scalar_mul(hv[:, :, i + 1], srcv, 0.5)
            for i in range(6):
                nc.scalar.activation(out=tv[:, :, i], in_=hv[:, :, i], func=ACTFN.Sin)

            # products -> pair slots (v0,v0, -v1,v1, v2,v2, -v3,v3), fp16
            eA = ec[:, 0:ETILES]
            ch, sh = tv[:, :, 0], tv[:, :, 1]
            ca, sa = tv[:, :, 2], tv[:, :, 3]
            cb, sb = tv[:, :, 4], tv[:, :, 5]

            def u1(x):
                return x.rearrange("p t (g two) -> p t g two", two=1)

            def b2(x):
                return u1(x).to_broadcast([P, ETILES, 24, 2])

            TT = nc.vector.tensor_tensor
            TS = nc.vector.tensor_scalar_mul
            TT(out=eA[:, :, :, 0:2], in0=b2(ch), in1=b2(ca), op=ALU.mult)
            TT(out=eA[:, :, :, 3:4], in0=u1(ch), in1=u1(sa), op=ALU.mult)
            TS(eA[:, :, :, 2:3], eA[:, :, :, 3:4], -1.0)
            TT(out=eA[:, :, :, 4:6], in0=b2(sh), in1=b2(cb), op=ALU.mult)
            TT(out=eA[:, :, :, 7:8], in0=u1(sh), in1=u1(sb), op=ALU.mult)
            TS(eA[:, :, :, 6:7], eA[:, :, :, 7:8], -1.0)

            # layer-0 |+> doubling factors; step k expands wire 5-k (gate 5-k)
            fA = fac[:, 0:ETILES]
            rev = eA[:, :, 5::-1, :]
            v0r, v1r = rev[:, :, :, 0:1], rev[:, :, :, 3:4]
            v2r, v3r = rev[:, :, :, 4:5], rev[:, :, :, 7:8]
            tmp6 = cp.tile([P, ETILES, 6, 1], F32)
            t6 = tmp6[:]

            def fpair(dst_lo, n, scale):
                src = t6.to_broadcast([P, ETILES, 6, n]) if n == 2 else t6
                TS(fA[:, :, :, dst_lo : dst_lo + n], src, scale)

            TT(out=t6, in0=v0r, in1=v2r, op=ALU.subtract)
            fpair(0, 2, R2)                       # (f0r, f0r)
            TT(out=t6, in0=v1r, in1=v3r, op=ALU.add)
            fpair(2, 1, R2)                       # -f0i  (f0i = -(v1+v3)*r2)
            fpair(3, 1, -R2)                      # +f0i
            TT(out=t6, in0=v0r, in1=v2r, op=ALU.add)
            fpair(4, 2, R2)                       # (f1r, f1r)
            TT(out=t6, in0=v1r, in1=v3r, op=ALU.subtract)
            fpair(6, 1, -R2)                      # -f1i  (f1i = (v1-v3)*r2)
            fpair(7, 1, R2)                       # +f1i

            # ---------------- phase A evolution ----------------
            stA = cp.tile([P, ETILES, P], F16)
            _emit_doubling(nc, gp, stA[:], fac[:, 0:ETILES], ETILES, "adb")
            for g, (c, t) in enumerate(CROTS):
                _emit_crot(nc, gp, stA[:], ec[:, 0:ETILES], ETILES, 6 + g, c, t, "ac")

            # T store + AllGather
            nc.sync.dma_start(
                out=T_loc[:].rearrange("(t p) k -> p t k", p=P), in_=stA[:]
            )
            if no_collective:
                nc.sync.dma_start(out=T_full[0:EPC, :], in_=T_loc[:])
            else:
                nc.gpsimd.collective_compute(
                    "AllGather",
                    ALU.bypass,
                    ins=[T_loc[:]],
                    outs=[T_full[:]],
                    replica_groups=[list(range(NCORES))],
                )

            # ---------------- phase W evolution ----------------
            stW = cp.tile([P, WTILES, P], F16)
            _emit_doubling(nc, gp, stW[:], fac[:, ETILES:], WTILES, "wdb")
            for g, (c, t) in enumerate(CROTS):
                _emit_crot(nc, gp, stW[:], ec[:, ETILES:], WTILES, 6 + g, c, t, "wc")

            # W^T slots into SBUF: even rows = basis states, odd = swap-negate
            tempw = cp.tile([P, WTILES, P], F16)
            sw_in = stW[:].rearrange("p n (w two) -> p (n w) two", two=2)[:, :, ::-1]
            pmb = pm[:].rearrange("p (o two) -> p o two", o=1).to_broadcast(
                [P, WTILES * NA, 2]
            )
            nc.vector.tensor_tensor(
                out=tempw[:].rearrange("p n (w two) -> p (n w) two", two=2),
                in0=sw_in, in1=pmb, op=ALU.mult,
            )
            wl = W_loc[:].rearrange("wt h j two k -> (h j) wt two k")
            nc.sync.dma_start(out=wl[:, :, 0, :], in_=stW[:])
            nc.sync.dma_start(out=wl[:, :, 1, :], in_=tempw[:])
            wsb = cp.tile([P, RSLOT, P], F16)
            nc.sync.dma_start(
                out=wsb[:],
                in_=W_loc[:].rearrange("wt h j two k -> (j two) (wt h) k"),
            )

            # ---------------- phase C ----------------
            scores = cp.tile([P, NT], F32)
            T_flat = T_full[:]
            for t in range(NT):
                gts = gtsp.tile([P, P], F16, tag="gts")
                nc.gpsimd.indirect_dma_start(
                    out=gts[:], out_offset=None, in_=T_flat,
                    in_offset=bass.IndirectOffsetOnAxis(
                        ap=sidx[:, t : t + 1], axis=0),
                )
                gto = gtop.tile([P, P], F16, tag="gto")
                nc.gpsimd.indirect_dma_start(
                    out=gto[:], out_offset=None, in_=T_flat,
                    in_offset=bass.IndirectOffsetOnAxis(
                        ap=oidx[:, t : t + 1], axis=0),
                )
                tst = tstp.tile([P, P], F16, tag="tst")
                eng = nc.sync if (t % 2 == 0) else nc.scalar
                eng.dma_start(out=tst[:], in_=gts[:], transpose=True)
                pY = psY.tile([P, P], F32, tag="py")
                nc.tensor.matmul(
                    out=pY[:], lhsT=tst[:], rhs=wsb[:, t // TPS, :],
                    start=True, stop=True,
                )
                scr = scrp.tile([P, P], F32, tag="scr")
                nc.vector.tensor_tensor(
                    out=scr[:], in0=gto[:], in1=pY[:], op=ALU.mult
                )
                sdum = scrp.tile([P, P], F32, tag="sdum")
                nc.scalar.activation(
                    out=sdum[:], in_=scr[:], func=ACTFN.Copy,
                    accum_out=scores[:, t : t + 1],
                )

            nc.sync.dma_start(out=scores_d[:], in_=scores[:])

    nc.finalize()
    return nc


# --------------------------------------------------------------------------
# host side
# --------------------------------------------------------------------------


def _rot_elems(params):
    """params [..., 3] (phi, theta, omega) -> v0, v1, v2, v3 arrays.

    m00=(v0,-v1) m01=(-v2,-v3) m10=(v2,-v3) m11=(v0,v1)
    """
    phi, tha, omg = params[..., 0], params[..., 1], params[..., 2]
    ch, sh = np.cos(tha / 2), np.sin(tha / 2)
    a, b = (phi + omg) / 2, (phi - omg) / 2
    return ch * np.cos(a), ch * np.sin(a), sh * np.cos(b), sh * np.sin(b)


def _host_prep(entity_params, relation_params, s_idx, p_idx, o_idx):
    ent = np.asarray(entity_params, dtype=np.float32)
    rel = np.asarray(relation_params, dtype=np.float32)
    s_idx = np.asarray(s_idx)
    p_idx = np.asarray(p_idx)
    o_idx = np.asarray(o_idx)

    # ---- entity shards ----
    ent_flat = ent.reshape(E, 72)
    ent_pad = np.zeros((EPAD, 72), np.float32)
    ent_pad[:E] = ent_flat
    ent_shards = [
        ent_pad[c * EPC : (c + 1) * EPC].reshape(ETILES, P, 72) for c in range(NCORES)
    ]

    # ---- p-sorted stream, contiguous core slices ----
    order = np.argsort(p_idx, kind="stable")
    per = B // NCORES
    gate_rel = rel.reshape(R, 24, 3)
    v0, v1, v2, v3 = _rot_elems(gate_rel)  # each [R, 24]

    in_maps = []
    outpos = np.full((NCORES, NT, P), -1, np.int64)
    for c in range(NCORES):
        sl = order[c * per : (c + 1) * per]
        rels_c = p_idx[sl]
        rels, starts = np.unique(rels_c, return_index=True)
        assert len(rels) <= RSLOT, f"core {c} has {len(rels)} relations"

        sidx = np.zeros((P, NT), np.int32)
        oidx = np.zeros((P, NT), np.int32)
        for s, r in enumerate(rels):
            elems = sl[rels_c == r]
            assert len(elems) <= TPS * P, f"relation {r} needs >3 tiles"
            for k in range(TPS):
                seg = elems[k * P : (k + 1) * P]
                n = len(seg)
                if n == 0:
                    break
                t = s * TPS + k
                sidx[:n, t] = s_idx[seg]
                oidx[:n, t] = o_idx[seg]
                outpos[c, t, :n] = seg

        # W coeff pair table [P, WTILES, 24, 8]
        wcoef = np.zeros((P, WTILES, 24, 8), np.float16)
        wfac = np.zeros((P, WTILES, 6, 8), np.float16)
        j = np.arange(NA)
        for s, r in enumerate(rels):
            wt, hhalf = divmod(s, 2)
            rows = slice(NA * hhalf, NA * hhalf + NA)
            for g in range(24):
                vals = (v0[r, g], v0[r, g], -v1[r, g], v1[r, g],
                        v2[r, g], v2[r, g], -v3[r, g], v3[r, g])
                for kk, vv in enumerate(vals):
                    wcoef[rows, wt, g, kk] = vv
            # basis doubling factors: step k expands wire 5-k; basis j bit
            # of wire q=5-k is (j >> k) & 1
            for k in range(6):
                qg = 5 - k
                bit = (j >> k) & 1
                m00 = (v0[r, qg], -v1[r, qg])
                m01 = (-v2[r, qg], -v3[r, qg])
                m10 = (v2[r, qg], -v3[r, qg])
                m11 = (v0[r, qg], v1[r, qg])
                wfac[rows, wt, k, 0] = np.where(bit == 0, m00[0], m01[0])
                wfac[rows, wt, k, 1] = wfac[rows, wt, k, 0]
                u0i = np.where(bit == 0, m00[1], m01[1])
                wfac[rows, wt, k, 2] = -u0i
                wfac[rows, wt, k, 3] = u0i
                wfac[rows, wt, k, 4] = np.where(bit == 0, m10[0], m11[0])
                wfac[rows, wt, k, 5] = wfac[rows, wt, k, 4]
                u1i = np.where(bit == 0, m10[1], m11[1])
                wfac[rows, wt, k, 6] = -u1i
                wfac[rows, wt, k, 7] = u1i

        in_maps.append(
            {
                "ent_par": ent_shards[c],
                "wcoef": wcoef,
                "wfac": wfac,
                "sidx": sidx,
                "oidx": oidx,
            }
        )
    return in_maps, outpos


_PROGRAM = None


def kernel(entity_params, relation_params, s_idx, p_idx, o_idx):
    global _PROGRAM
    in_maps, outpos = _host_prep(entity_params, relation_params, s_idx, p_idx, o_idx)
    if _PROGRAM is None:
        _PROGRAM = build_program()
    nc = _PROGRAM
    res = run_bass_kernel_spmd(nc, in_maps, list(range(NCORES)))
    out = np.zeros(B, np.float32)
    for c in range(NCORES):
        sc = res.results[c]["scores"]  # [P, NT]
        pos = outpos[c]  # [NT, P]
        mask = pos >= 0
        out[pos[mask]] = sc.T[mask]
    return out


if __name__ == "__main__":
    build_program()
    print("build OK")


# revision 26
# speedup vs baseline: 1.6097x; 1.5765x over previous
"""Trainium2 Bass kernel for the quantum-circuit KG-embedding scoring model.

Math: score(s,p,o) = Re(<B_o h | W_p | B_s h>) where B_e / W_p are the
24-gate circuit blocks for entity/relation params and h = |+>^6.

v4 design (fp16, interleaved re/im pairs):
  State layout [128 part, nt, 128] fp16 where the 128 free elems are 64
  amplitudes as interleaved (re, im) pairs.  A 2x2 gate update is 14
  tensor_tensor ops (8 pair-coeff products + 6 adds; re/im swaps are
  negative-stride reads of contiguous temps), all DVE "2x_1p" eligible.

  Phase A: evolve 1280 local entities (10 tiles); store fp16 rows,
    AllGather -> T_full [10240, 128].
  Phase W: evolve 16 W tiles (2 relations x 64 basis cols each) on DVE
    WHILE phase C's gathers run on the Pool engine; W^T slots assembled
    into SBUF wsb [128, 32, 128] via 4 partition-strided SBUF->SBUF DMAs
    (even rows = basis states, odd rows = swap-negate).
  Phase C: host packs the p-sorted batch into a STATIC slot->tile map
    (slot s = t//3; every relation on a core needs <= 3 tiles, <= 32
    relations per core - verified for B=65536, R=200).  Per tile: two
    single-index indirect row-gathers (s, o), XBAR DMA transpose of Ts,
    PE matmul Y^T = Ts @ W^T_slot (static SBUF rhs), DVE product with
    To, ACT-engine accumulate into scores.

Host does only: trig for the 200-relation coeff tables (tiny), index
sort/packing, and output unpermute.
"""

import sys
import numpy as np

for _p in ("/opt/trn_rl_repo",):
    if _p not in sys.path:
        sys.path.insert(0, _p)

import concourse.bass as bass
import concourse.bacc as bacc
import concourse.mybir as mybir
from concourse import tile
from concourse.bass_utils import run_bass_kernel_spmd

F32 = mybir.dt.float32
F16 = mybir.dt.float16
I32 = mybir.dt.int32
ALU = mybir.AluOpType
ACTFN = mybir.ActivationFunctionType

P = 128
Q = 6
NA = 64                      # 2^Q amplitudes
NCORES = 8
E, R, B = 10000, 200, 65536
ETILES = 10                  # entity tiles per core
EPC = ETILES * P             # 1280 entities per core
EPAD = EPC * NCORES          # 10240 padded entity rows
WTILES = 16                  # W-phase tiles per core (2 relations each)
RSLOT = 2 * WTILES           # 32 relation slots per core
TPS = 3                      # static tiles per slot
NT = RSLOT * TPS             # 96 phase-C tiles per core
R2 = float(2.0 ** -0.5)
PI = float(np.pi)

# CRot gate list: (control, target) wire pairs, in circuit order
CROTS = [(q, (q + off) % Q) for off in (1, 2, 3) for q in range(Q)]


# --------------------------------------------------------------------------
# device program
# --------------------------------------------------------------------------


def _pair_bc(coef_ap, nt, nrep):
    """[p, nt, 2] pair slice -> broadcast to [p, nt, nrep, 2] (4-D)."""
    v = coef_ap.rearrange("p n (m two) -> p n m two", m=1, two=2)
    return v.to_broadcast([P, nt, nrep, 2])


def _emit_crot(nc, pool, st, ec, nt, g, c, t, tag):
    """Apply CRot gate g (control c, target t) in place on st [P, nt, 128].

    ec: [P, nt, 24, 8] fp16 pair-coeff slots (v0,v0, -v1,v1, v2,v2, -v3,v3).
    """
    cpos, tpos = 5 - c, 5 - t
    hi, lo = max(cpos, tpos), min(cpos, tpos)
    A = 1 << (5 - hi)
    Bm = 1 << (hi - lo - 1)
    C = 1 << lo
    v = st.rearrange(
        "p n (a x b y c two) -> p n a x b y c two",
        a=A, x=2, b=Bm, y=2, c=C, two=2,
    )
    cbit_is_x = cpos == hi

    def sel(cv, tv):
        xv, yv = (cv, tv) if cbit_is_x else (tv, cv)
        return v[:, :, :, xv, :, yv, :, :]

    a0 = sel(1, 0)   # [p, n, A, Bm, C, 2]
    a1 = sel(1, 1)
    L = A * Bm * C * 2  # 32

    def co(lo_s):
        return _pair_bc(ec[:, :, g, lo_s : lo_s + 2], nt, A * Bm * C)

    def qt(tg):
        tt = pool.tile([P, nt, L], F16, tag=tag + tg)
        return tt

    def as5(tt):
        return tt[:].rearrange(
            "p n (a b c two) -> p n a b c two", a=A, b=Bm, c=C, two=2
        )

    def flat(tt):
        return tt[:].rearrange("p n (l two) -> p n l two", two=2)

    def swap(tt):
        return flat(tt)[:, :, :, ::-1]

    c0, c1, c2, c3 = co(0), co(2), co(4), co(6)
    q1, q2, q3, q4 = qt("q1"), qt("q2"), qt("q3"), qt("q4")
    q5, q6, q7, q8 = qt("q5"), qt("q6"), qt("q7"), qt("q8")
    TT = nc.vector.tensor_tensor
    TT(out=as5(q1), in0=a0, in1=c0, op=ALU.mult)
    TT(out=as5(q2), in0=a0, in1=c1, op=ALU.mult)
    TT(out=as5(q3), in0=a1, in1=c2, op=ALU.mult)
    TT(out=as5(q4), in0=a1, in1=c3, op=ALU.mult)
    TT(out=as5(q5), in0=a0, in1=c2, op=ALU.mult)
    TT(out=as5(q6), in0=a0, in1=c3, op=ALU.mult)
    TT(out=as5(q7), in0=a1, in1=c0, op=ALU.mult)
    TT(out=as5(q8), in0=a1, in1=c1, op=ALU.mult)
    pa, pb = qt("pa"), qt("pb")
    pc, pd = qt("pc"), qt("pd")
    TT(out=flat(pa), in0=flat(q1), in1=swap(q2), op=ALU.add)
    TT(out=flat(pb), in0=flat(q3), in1=swap(q4), op=ALU.subtract)
    TT(out=flat(pc), in0=flat(q5), in1=swap(q6), op=ALU.add)
    TT(out=flat(pd), in0=flat(q7), in1=swap(q8), op=ALU.subtract)
    TT(out=a0, in0=as5(pa), in1=as5(pb), op=ALU.subtract)
    TT(out=a1, in0=as5(pc), in1=as5(pd), op=ALU.add)


def _pair_bc_db(coef_ap, nt, w):
    v = coef_ap.rearrange("p n (w two) -> p n w two", w=1, two=2)
    return v.to_broadcast([P, nt, w, 2])


def _emit_doubling(nc, pool, st, fac, nt, tag):
    """Product-state doubling, in place on st [P, nt, 128] fp16.

    fac [P, nt, 6, 8] fp16 slots per step: (f0r,f0r, -f0i,f0i, f1r,f1r,
    -f1i,f1i); step k expands amplitude bit k (wire 5-k).
    """
    CP = nc.vector.tensor_copy
    TT = nc.vector.tensor_tensor
    # seed from step-0 factors: amp0 = f0, amp1 = f1
    CP(out=st[:, :, 0:1], in_=fac[:, :, 0, 0:1])
    CP(out=st[:, :, 1:2], in_=fac[:, :, 0, 3:4])
    CP(out=st[:, :, 2:3], in_=fac[:, :, 0, 4:5])
    CP(out=st[:, :, 3:4], in_=fac[:, :, 0, 7:8])
    for k in range(1, 6):
        w = 1 << k  # current state width in pairs
        cview = st[:, :, 0 : 2 * w].rearrange("p n (w two) -> p n w two", two=2)
        for m in (1, 0):  # m=1 writes fresh upper half first
            frp = _pair_bc_db(fac[:, :, k, 4 * m : 4 * m + 2], nt, w)
            fim = _pair_bc_db(fac[:, :, k, 4 * m + 2 : 4 * m + 4], nt, w)
            t1 = pool.tile([P, nt, 2 * w], F16, tag=tag + "A")
            t2 = pool.tile([P, nt, 2 * w], F16, tag=tag + "B")
            t1v = t1[:].rearrange("p n (w two) -> p n w two", two=2)
            t2v = t2[:].rearrange("p n (w two) -> p n w two", two=2)
            TT(out=t1v, in0=cview, in1=frp, op=ALU.mult)
            TT(out=t2v, in0=cview, in1=fim, op=ALU.mult)
            t1f = t1[:].rearrange("p n (w two) -> p n w two", two=2)
            t2s = t2[:].rearrange("p n (w two) -> p n w two", two=2)[:, :, :, ::-1]
            df = st[:, :, m * 2 * w : (m + 1) * 2 * w].rearrange(
                "p n (w two) -> p n w two", two=2
            )
            TT(out=df, in0=t1f, in1=t2s, op=ALU.subtract)


def build_program(no_collective=False):
    nc = bacc.Bacc("TRN2", target_bir_lowering=False, debug=False)

    ent = nc.dram_tensor("ent_par", [ETILES, P, 72], F32, kind="ExternalInput")
    wcoef_d = nc.dram_tensor("wcoef", [P, WTILES, 24, 8], F16, kind="ExternalInput")
    wfac_d = nc.dram_tensor("wfac", [P, WTILES, 6, 8], F16, kind="ExternalInput")
    sidx_d = nc.dram_tensor("sidx", [P, NT], I32, kind="ExternalInput")
    oidx_d = nc.dram_tensor("oidx", [P, NT], I32, kind="ExternalInput")
    ident_d = nc.dram_tensor("ident", [P, P], F16, kind="ExternalInput")
    scores_d = nc.dram_tensor("scores", [P, NT], F32, kind="ExternalOutput")

    with tile.TileContext(nc) as tc:
        with (
            tc.tile_pool(name="const", bufs=1) as cp,
            tc.tile_pool(name="gtmp", bufs=2) as gp,
            tc.tile_pool(name="gts", bufs=6) as gtsp,
            tc.tile_pool(name="gto", bufs=6) as gtop,
            tc.tile_pool(name="tst", bufs=6) as tstp,
            tc.tile_pool(name="scr", bufs=4) as scrp,
            tc.tile_pool(name="cpy", bufs=4, space="PSUM") as psY,
            tc.tile_pool(name="dram", bufs=1, space="DRAM") as dp,
        ):
            # ---------------- DRAM scratch ----------------
            T_loc = dp.tile([EPC, P], F16)
            T_full = dp.tile([EPAD, P], F16, addr_space="Shared")
            W_loc = dp.tile([WTILES, 2, NA, 2, P], F16)

            # ---------------- load inputs ----------------
            ang = cp.tile([P, ETILES, 72], F32)
            nc.sync.dma_start(out=ang[:], in_=ent[:].rearrange("t p k -> p t k"))
            ec = cp.tile([P, ETILES + WTILES, 24, 8], F16)
            fac = cp.tile([P, ETILES + WTILES, 6, 8], F16)
            nc.sync.dma_start(out=ec[:, ETILES:], in_=wcoef_d[:])
            nc.sync.dma_start(out=fac[:, ETILES:], in_=wfac_d[:])
            sidx = cp.tile([P, NT], I32)
            nc.sync.dma_start(out=sidx[:], in_=sidx_d[:])
            oidx = cp.tile([P, NT], I32)
            nc.sync.dma_start(out=oidx[:], in_=oidx_d[:])
            ident = cp.tile([P, P], F16)
            nc.sync.dma_start(out=ident[:], in_=ident_d[:])

            pm = cp.tile([P, 2], F16)
            nc.vector.memset(pm[:, 0:1], -1.0)
            nc.vector.memset(pm[:, 1:2], 1.0)

            cdb = cp.tile([P, 3], F32)
            nc.vector.memset(cdb[:, 0:1], 0.0)
            nc.vector.memset(cdb[:, 1:2], 0.5)
            nc.vector.memset(cdb[:, 2:3], PI / 2)
            nc.const_aps.aps[(F32, 0.0)] = cdb[:, 0:1]
            nc.const_aps.aps[(F32, 0.5)] = cdb[:, 1:2]
            nc.const_aps.aps[(F32, PI / 2)] = cdb[:, 2:3]

            # ---------------- entity coeffs (A part) ----------------
            av = ang[:].rearrange("p t (g a) -> p t g a", g=24, a=3)
            phi, tha, omg = av[:, :, :, 0], av[:, :, :, 1], av[:, :, :, 2]
            s1 = cp.tile([P, ETILES, 24], F32)
            s2 = cp.tile([P, ETILES, 24], F32)
            nc.vector.tensor_tensor(out=s1[:], in0=phi, in1=omg, op=ALU.add)
            nc.vector.tensor_tensor(out=s2[:], in0=phi, in1=omg, op=ALU.subtract)

            half = cp.tile([P, ETILES, 6, 24], F32)
            trig = cp.tile([P, ETILES, 6, 24], F32)
            hv, tv = half[:], trig[:]
            for i, srcv in ((0, tha), (2, s1[:]), (4, s2[:])):
                nc.vector.tensor_scalar(
                    out=hv[:, :, i], in0=srcv, scalar1=0.5, scalar2=PI / 2,
                    op0=ALU.mult, op1=ALU.add,
                )
                nc.vector.tensor_scalar_mul(hv[:, :, i + 1], srcv, 0.5)
            for i in range(6):
                nc.scalar.activation(out=tv[:, :, i], in_=hv[:, :, i], func=ACTFN.Sin)

            # products -> pair slots (v0,v0, -v1,v1, v2,v2, -v3,v3), fp16
            eA = ec[:, 0:ETILES]
            ch, sh = tv[:, :, 0], tv[:, :, 1]
            ca, sa = tv[:, :, 2], tv[:, :, 3]
            cb, sb = tv[:, :, 4], tv[:, :, 5]

            def u1(x):
                return x.rearrange("p t (g two) -> p t g two", two=1)

            def b2(x):
                return u1(x).to_broadcast([P, ETILES, 24, 2])

            TT = nc.vector.tensor_tensor
            TS = nc.vector.tensor_scalar_mul
            TT(out=eA[:, :, :, 0:2], in0=b2(ch), in1=b2(ca), op=ALU.mult)
            TT(out=eA[:, :, :, 3:4], in0=u1(ch), in1=u1(sa), op=ALU.mult)
            TS(eA[:, :, :, 2:3], eA[:, :, :, 3:4], -1.0)
            TT(out=eA[:, :, :, 4:6], in0=b2(sh), in1=b2(cb), op=ALU.mult)
            TT(out=eA[:, :, :, 7:8], in0=u1(sh), in1=u1(sb), op=ALU.mult)
            TS(eA[:, :, :, 6:7], eA[:, :, :, 7:8], -1.0)

            # layer-0 |+> doubling factors; step k expands wire 5-k (gate 5-k)
            fA = fac[:, 0:ETILES]
            rev = eA[:, :, 5::-1, :]
            v0r, v1r = rev[:, :, :, 0:1], rev[:, :, :, 3:4]
            v2r, v3r = rev[:, :, :, 4:5], rev[:, :, :, 7:8]
            tmp6 = cp.tile([P, ETILES, 6, 1], F32)
            t6 = tmp6[:]

            def fpair(dst_lo, n, scale):
                src = t6.to_broadcast([P, ETILES, 6, n]) if n == 2 else t6
                TS(fA[:, :, :, dst_lo : dst_lo + n], src, scale)

            TT(out=t6, in0=v0r, in1=v2r, op=ALU.subtract)
            fpair(0, 2, R2)                       # (f0r, f0r)
            TT(out=t6, in0=v1r, in1=v3r, op=ALU.add)
            fpair(2, 1, R2)                       # -f0i  (f0i = -(v1+v3)*r2)
            fpair(3, 1, -R2)                      # +f0i
            TT(out=t6, in0=v0r, in1=v2r, op=ALU.add)
            fpair(4, 2, R2)                       # (f1r, f1r)
            TT(out=t6, in0=v1r, in1=v3r, op=ALU.subtract)
            fpair(6, 1, -R2)                      # -f1i  (f1i = (v1-v3)*r2)
            fpair(7, 1, R2)                       # +f1i

            # ---------------- phase A evolution ----------------
            stA = cp.tile([P, ETILES, P], F16)
            _emit_doubling(nc, gp, stA[:], fac[:, 0:ETILES], ETILES, "adb")
            for g, (c, t) in enumerate(CROTS):
                _emit_crot(nc, gp, stA[:], ec[:, 0:ETILES], ETILES, 6 + g, c, t, "ac")

            # T store + AllGather
            nc.sync.dma_start(
                out=T_loc[:].rearrange("(t p) k -> p t k", p=P), in_=stA[:]
            )
            if no_collective:
                nc.sync.dma_start(out=T_full[0:EPC, :], in_=T_loc[:])
            else:
                nc.gpsimd.collective_compute(
                    "AllGather",
                    ALU.bypass,
                    ins=[T_loc[:]],
                    outs=[T_full[:]],
                    replica_groups=[list(range(NCORES))],
                )

            # ---------------- phase W evolution ----------------
            stW = cp.tile([P, WTILES, P], F16)
            _emit_doubling(nc, gp, stW[:], fac[:, ETILES:], WTILES, "wdb")
            for g, (c, t) in enumerate(CROTS):
                _emit_crot(nc, gp, stW[:], ec[:, ETILES:], WTILES, 6 + g, c, t, "wc")

            # W^T slots into SBUF: even rows = basis states, odd = swap-negate
            tempw = cp.tile([P, WTILES, P], F16)
            sw_in = stW[:].rearrange("p n (w two) -> p (n w) two", two=2)[:, :, ::-1]
            pmb = pm[:].rearrange("p (o two) -> p o two", o=1).to_broadcast(
                [P, WTILES * NA, 2]
            )
            nc.vector.tensor_tensor(
                out=tempw[:].rearrange("p n (w two) -> p (n w) two", two=2),
                in0=sw_in, in1=pmb, op=ALU.mult,
            )
            wl = W_loc[:].rearrange("wt h j two k -> (h j) wt two k")
            nc.sync.dma_start(out=wl[:, :, 0, :], in_=stW[:])
            nc.sync.dma_start(out=wl[:, :, 1, :], in_=tempw[:])
            wsb = cp.tile([P, RSLOT, P], F16)
            nc.sync.dma_start(
                out=wsb[:],
                in_=W_loc[:].rearrange("wt h j two k -> (j two) (wt h) k"),
            )

            # ---------------- phase C ----------------
            scores = cp.tile([P, NT], F32)
            T_flat = T_full[:]
            for t in range(NT):
                gts = gtsp.tile([P, P], F16, tag="gts")
                nc.gpsimd.indirect_dma_start(
                    out=gts[:], out_offset=None, in_=T_flat,
                    in_offset=bass.IndirectOffsetOnAxis(
                        ap=sidx[:, t : t + 1], axis=0),
                )
                gto = gtop.tile([P, P], F16, tag="gto")
                nc.gpsimd.indirect_dma_start(
                    out=gto[:], out_offset=None, in_=T_flat,
                    in_offset=bass.IndirectOffsetOnAxis(
                        ap=oidx[:, t : t + 1], axis=0),
                )
                psT = psY.tile([P, P], F32, tag="pst")
                nc.tensor.matmul(
                    out=psT[:], lhsT=gts[:], rhs=ident[:], start=True, stop=True
                )
                tst = tstp.tile([P, P], F16, tag="tst")
                nc.vector.tensor_copy(out=tst[:], in_=psT[:])
                pY = psY.tile([P, P], F32, tag="py")
                nc.tensor.matmul(
                    out=pY[:], lhsT=tst[:], rhs=wsb[:, t // TPS, :],
                    start=True, stop=True,
                )
                scr = scrp.tile([P, P], F32, tag="scr")
                nc.vector.tensor_tensor(
                    out=scr[:], in0=gto[:], in1=pY[:], op=ALU.mult
                )
                sdum = scrp.tile([P, P], F32, tag="sdum")
                nc.scalar.activation(
                    out=sdum[:], in_=scr[:], func=ACTFN.Copy,
                    accum_out=scores[:, t : t + 1],
                )

            nc.sync.dma_start(out=scores_d[:], in_=scores[:])

    nc.finalize()
    return nc


# --------------------------------------------------------------------------
# host side
# --------------------------------------------------------------------------


def _rot_elems(params):
    """params [..., 3] (phi, theta, omega) -> v0, v1, v2, v3 arrays.

    m00=(v0,-v1) m01=(-v2,-v3) m10=(v2,-v3) m11=(v0,v1)
    """
    phi, tha, omg = params[..., 0], params[..., 1], params[..., 2]
    ch, sh = np.cos(tha / 2), np.sin(tha / 2)
    a, b = (phi + omg) / 2, (phi - omg) / 2
    return ch * np.cos(a), ch * np.sin(a), sh * np.cos(b), sh * np.sin(b)


def _host_prep(entity_params, relation_params, s_idx, p_idx, o_idx):
    ent = np.asarray(entity_params, dtype=np.float32)
    rel = np.asarray(relation_params, dtype=np.float32)
    s_idx = np.asarray(s_idx)
    p_idx = np.asarray(p_idx)
    o_idx = np.asarray(o_idx)

    # ---- entity shards ----
    ent_flat = ent.reshape(E, 72)
    ent_pad = np.zeros((EPAD, 72), np.float32)
    ent_pad[:E] = ent_flat
    ent_shards = [
        ent_pad[c * EPC : (c + 1) * EPC].reshape(ETILES, P, 72) for c in range(NCORES)
    ]

    # ---- p-sorted stream, contiguous core slices ----
    order = np.argsort(p_idx, kind="stable")
    per = B // NCORES
    gate_rel = rel.reshape(R, 24, 3)
    v0, v1, v2, v3 = _rot_elems(gate_rel)  # each [R, 24]

    in_maps = []
    outpos = np.full((NCORES, NT, P), -1, np.int64)
    for c in range(NCORES):
        sl = order[c * per : (c + 1) * per]
        rels_c = p_idx[sl]
        rels, starts = np.unique(rels_c, return_index=True)
        assert len(rels) <= RSLOT, f"core {c} has {len(rels)} relations"

        sidx = np.zeros((P, NT), np.int32)
        oidx = np.zeros((P, NT), np.int32)
        for s, r in enumerate(rels):
            elems = sl[rels_c == r]
            assert len(elems) <= TPS * P, f"relation {r} needs >3 tiles"
            for k in range(TPS):
                seg = elems[k * P : (k + 1) * P]
                n = len(seg)
                if n == 0:
                    break
                t = s * TPS + k
                sidx[:n, t] = s_idx[seg]
                oidx[:n, t] = o_idx[seg]
                outpos[c, t, :n] = seg

        # W coeff pair table [P, WTILES, 24, 8]
        wcoef = np.zeros((P, WTILES, 24, 8), np.float16)
        wfac = np.zeros((P, WTILES, 6, 8), np.float16)
        j = np.arange(NA)
        for s, r in enumerate(rels):
            wt, hhalf = divmod(s, 2)
            rows = slice(NA * hhalf, NA * hhalf + NA)
            for g in range(24):
                vals = (v0[r, g], v0[r, g], -v1[r, g], v1[r, g],
                        v2[r, g], v2[r, g], -v3[r, g], v3[r, g])
                for kk, vv in enumerate(vals):
                    wcoef[rows, wt, g, kk] = vv
            # basis doubling factors: step k expands wire 5-k; basis j bit
            # of wire q=5-k is (j >> k) & 1
            for k in range(6):
                qg = 5 - k
                bit = (j >> k) & 1
                m00 = (v0[r, qg], -v1[r, qg])
                m01 = (-v2[r, qg], -v3[r, qg])
                m10 = (v2[r, qg], -v3[r, qg])
                m11 = (v0[r, qg], v1[r, qg])
                wfac[rows, wt, k, 0] = np.where(bit == 0, m00[0], m01[0])
                wfac[rows, wt, k, 1] = wfac[rows, wt, k, 0]
                u0i = np.where(bit == 0, m00[1], m01[1])
                wfac[rows, wt, k, 2] = -u0i
                wfac[rows, wt, k, 3] = u0i
                wfac[rows, wt, k, 4] = np.where(bit == 0, m10[0], m11[0])
                wfac[rows, wt, k, 5] = wfac[rows, wt, k, 4]
                u1i = np.where(bit == 0, m10[1], m11[1])
                wfac[rows, wt, k, 6] = -u1i
                wfac[rows, wt, k, 7] = u1i

        in_maps.append(
            {
                "ent_par": ent_shards[c],
                "wcoef": wcoef,
                "wfac": wfac,
                "sidx": sidx,
                "oidx": oidx,
                "ident": np.eye(P, dtype=np.float16),
            }
        )
    return in_maps, outpos


_PROGRAM = None


def kernel(entity_params, relation_params, s_idx, p_idx, o_idx):
    global _PROGRAM
    in_maps, outpos = _host_prep(entity_params, relation_params, s_idx, p_idx, o_idx)
    if _PROGRAM is None:
        _PROGRAM = build_program()
    nc = _PROGRAM
    res = run_bass_kernel_spmd(nc, in_maps, list(range(NCORES)))
    out = np.zeros(B, np.float32)
    for c in range(NCORES):
        sc = res.results[c]["scores"]  # [P, NT]
        pos = outpos[c]  # [NT, P]
        mask = pos >= 0
        out[pos[mask]] = sc.T[mask]
    return out


if __name__ == "__main__":
    build_program()
    print("build OK")


# revision 29
# speedup vs baseline: 1.8457x; 1.1467x over previous
"""Trainium2 Bass kernel for the quantum-circuit KG-embedding scoring model.

Math: score(s,p,o) = Re(<B_o h | W_p | B_s h>) where B_e / W_p are the
24-gate circuit blocks for entity/relation params and h = |+>^6.

v4 design (fp16, interleaved re/im pairs):
  State layout [128 part, nt, 128] fp16 where the 128 free elems are 64
  amplitudes as interleaved (re, im) pairs.  A 2x2 gate update is 14
  tensor_tensor ops (8 pair-coeff products + 6 adds; re/im swaps are
  negative-stride reads of contiguous temps), all DVE "2x_1p" eligible.

  Phase A: evolve 1280 local entities (10 tiles); store fp16 rows,
    AllGather -> T_full [10240, 128].
  Phase W: evolve 16 W tiles (2 relations x 64 basis cols each) on DVE
    WHILE phase C's gathers run on the Pool engine; W^T slots assembled
    into SBUF wsb [128, 32, 128] via 4 partition-strided SBUF->SBUF DMAs
    (even rows = basis states, odd rows = swap-negate).
  Phase C: host packs the p-sorted batch into a STATIC slot->tile map
    (slot s = t//3; every relation on a core needs <= 3 tiles, <= 32
    relations per core - verified for B=65536, R=200).  Per tile: two
    single-index indirect row-gathers (s, o), XBAR DMA transpose of Ts,
    PE matmul Y^T = Ts @ W^T_slot (static SBUF rhs), DVE product with
    To, ACT-engine accumulate into scores.

Host does only: trig for the 200-relation coeff tables (tiny), index
sort/packing, and output unpermute.
"""

import sys
import numpy as np

for _p in ("/opt/trn_rl_repo",):
    if _p not in sys.path:
        sys.path.insert(0, _p)

import concourse.bass as bass
import concourse.bacc as bacc
import concourse.mybir as mybir
from concourse import tile
from concourse.bass_utils import run_bass_kernel_spmd

F32 = mybir.dt.float32
F16 = mybir.dt.float16
I32 = mybir.dt.int32
ALU = mybir.AluOpType
ACTFN = mybir.ActivationFunctionType

P = 128
Q = 6
NA = 64                      # 2^Q amplitudes
NCORES = 8
E, R, B = 10000, 200, 65536
ETILES = 10                  # entity tiles per core
EPC = ETILES * P             # 1280 entities per core
EPAD = EPC * NCORES          # 10240 padded entity rows
WTILES = 14                  # W-phase tiles per core (2 relations each)
RSLOT = 2 * WTILES           # 28 relation slots per core
TPS = 3                      # static tiles per slot
NT = RSLOT * TPS             # 96 phase-C tiles per core
R2 = float(2.0 ** -0.5)
PI = float(np.pi)

# CRot gate list: (control, target) wire pairs, in circuit order
CROTS = [(q, (q + off) % Q) for off in (1, 2, 3) for q in range(Q)]


# --------------------------------------------------------------------------
# device program
# --------------------------------------------------------------------------


def _pair_bc(coef_ap, nt, nrep):
    """[p, nt, 2] pair slice -> broadcast to [p, nt, nrep, 2] (4-D)."""
    v = coef_ap.rearrange("p n (m two) -> p n m two", m=1, two=2)
    return v.to_broadcast([P, nt, nrep, 2])


def _emit_crot(nc, pool, st, ec, nt, g, c, t, tag, eng=None):
    """Apply CRot gate g (control c, target t) in place on st [P, nt, 128].

    ec: [P, nt, 24, 8] fp16 pair-coeff slots (v0,v0, -v1,v1, v2,v2, -v3,v3).
    """
    cpos, tpos = 5 - c, 5 - t
    hi, lo = max(cpos, tpos), min(cpos, tpos)
    A = 1 << (5 - hi)
    Bm = 1 << (hi - lo - 1)
    C = 1 << lo
    v = st.rearrange(
        "p n (a x b y c two) -> p n a x b y c two",
        a=A, x=2, b=Bm, y=2, c=C, two=2,
    )
    cbit_is_x = cpos == hi

    def sel(cv, tv):
        xv, yv = (cv, tv) if cbit_is_x else (tv, cv)
        return v[:, :, :, xv, :, yv, :, :]

    a0 = sel(1, 0)   # [p, n, A, Bm, C, 2]
    a1 = sel(1, 1)
    L = A * Bm * C * 2  # 32

    def co(lo_s):
        return _pair_bc(ec[:, :, g, lo_s : lo_s + 2], nt, A * Bm * C)

    def qt(tg):
        tt = pool.tile([P, nt, L], F16, tag=tag + tg)
        return tt

    def as5(tt):
        return tt[:].rearrange(
            "p n (a b c two) -> p n a b c two", a=A, b=Bm, c=C, two=2
        )

    def flat(tt):
        return tt[:].rearrange("p n (l two) -> p n l two", two=2)

    def swap(tt):
        return flat(tt)[:, :, :, ::-1]

    c0, c1, c2, c3 = co(0), co(2), co(4), co(6)
    q1, q2, q3, q4 = qt("q1"), qt("q2"), qt("q3"), qt("q4")
    q5, q6, q7, q8 = qt("q5"), qt("q6"), qt("q7"), qt("q8")
    TT = (eng or nc.vector).tensor_tensor
    TT(out=as5(q1), in0=a0, in1=c0, op=ALU.mult)
    TT(out=as5(q2), in0=a0, in1=c1, op=ALU.mult)
    TT(out=as5(q3), in0=a1, in1=c2, op=ALU.mult)
    TT(out=as5(q4), in0=a1, in1=c3, op=ALU.mult)
    TT(out=as5(q5), in0=a0, in1=c2, op=ALU.mult)
    TT(out=as5(q6), in0=a0, in1=c3, op=ALU.mult)
    TT(out=as5(q7), in0=a1, in1=c0, op=ALU.mult)
    TT(out=as5(q8), in0=a1, in1=c1, op=ALU.mult)
    pa, pb = qt("pa"), qt("pb")
    pc, pd = qt("pc"), qt("pd")
    TT(out=flat(pa), in0=flat(q1), in1=swap(q2), op=ALU.add)
    TT(out=flat(pb), in0=flat(q3), in1=swap(q4), op=ALU.subtract)
    TT(out=flat(pc), in0=flat(q5), in1=swap(q6), op=ALU.add)
    TT(out=flat(pd), in0=flat(q7), in1=swap(q8), op=ALU.subtract)
    TT(out=a0, in0=as5(pa), in1=as5(pb), op=ALU.subtract)
    TT(out=a1, in0=as5(pc), in1=as5(pd), op=ALU.add)


def _pair_bc_db(coef_ap, nt, w):
    v = coef_ap.rearrange("p n (w two) -> p n w two", w=1, two=2)
    return v.to_broadcast([P, nt, w, 2])


def _emit_doubling(nc, pool, st, fac, nt, tag, eng=None):
    """Product-state doubling, in place on st [P, nt, 128] fp16.

    fac [P, nt, 6, 8] fp16 slots per step: (f0r,f0r, -f0i,f0i, f1r,f1r,
    -f1i,f1i); step k expands amplitude bit k (wire 5-k).
    """
    CP = (eng or nc.vector).tensor_copy
    TT = (eng or nc.vector).tensor_tensor
    # seed from step-0 factors: amp0 = f0, amp1 = f1
    CP(out=st[:, :, 0:1], in_=fac[:, :, 0, 0:1])
    CP(out=st[:, :, 1:2], in_=fac[:, :, 0, 3:4])
    CP(out=st[:, :, 2:3], in_=fac[:, :, 0, 4:5])
    CP(out=st[:, :, 3:4], in_=fac[:, :, 0, 7:8])
    for k in range(1, 6):
        w = 1 << k  # current state width in pairs
        cview = st[:, :, 0 : 2 * w].rearrange("p n (w two) -> p n w two", two=2)
        for m in (1, 0):  # m=1 writes fresh upper half first
            frp = _pair_bc_db(fac[:, :, k, 4 * m : 4 * m + 2], nt, w)
            fim = _pair_bc_db(fac[:, :, k, 4 * m + 2 : 4 * m + 4], nt, w)
            t1 = pool.tile([P, nt, 2 * w], F16, tag=tag + "A")
            t2 = pool.tile([P, nt, 2 * w], F16, tag=tag + "B")
            t1v = t1[:].rearrange("p n (w two) -> p n w two", two=2)
            t2v = t2[:].rearrange("p n (w two) -> p n w two", two=2)
            TT(out=t1v, in0=cview, in1=frp, op=ALU.mult)
            TT(out=t2v, in0=cview, in1=fim, op=ALU.mult)
            t1f = t1[:].rearrange("p n (w two) -> p n w two", two=2)
            t2s = t2[:].rearrange("p n (w two) -> p n w two", two=2)[:, :, :, ::-1]
            df = st[:, :, m * 2 * w : (m + 1) * 2 * w].rearrange(
                "p n (w two) -> p n w two", two=2
            )
            TT(out=df, in0=t1f, in1=t2s, op=ALU.subtract)


def build_program(no_collective=False):
    nc = bacc.Bacc("TRN2", target_bir_lowering=False, debug=False)

    ent = nc.dram_tensor("ent_par", [ETILES, P, 72], F32, kind="ExternalInput")
    wcoef_d = nc.dram_tensor("wcoef", [P, WTILES, 24, 8], F16, kind="ExternalInput")
    wfac_d = nc.dram_tensor("wfac", [P, WTILES, 6, 8], F16, kind="ExternalInput")
    sidx_d = nc.dram_tensor("sidx", [P, NT], I32, kind="ExternalInput")
    oidx_d = nc.dram_tensor("oidx", [P, NT], I32, kind="ExternalInput")
    ident_d = nc.dram_tensor("ident", [P, P], F16, kind="ExternalInput")
    scores_d = nc.dram_tensor("scores", [P, NT], F32, kind="ExternalOutput")

    with tile.TileContext(nc) as tc:
        with (
            tc.tile_pool(name="const", bufs=1) as cp,
            tc.tile_pool(name="gtmp", bufs=2) as gp,
            tc.tile_pool(name="gts", bufs=8) as gtsp,
            tc.tile_pool(name="gto", bufs=8) as gtop,
            tc.tile_pool(name="tst", bufs=8) as tstp,
            tc.tile_pool(name="scr", bufs=6) as scrp,
            tc.tile_pool(name="cpy", bufs=4, space="PSUM") as psY,
            tc.tile_pool(name="dram", bufs=1, space="DRAM") as dp,
        ):
            # ---------------- DRAM scratch ----------------
            T_loc = dp.tile([EPC, P], F16)
            T_full = dp.tile([EPAD, P], F16, addr_space="Shared")
            W_loc = dp.tile([WTILES, 2, NA, 2, P], F16)

            # ---------------- load inputs ----------------
            ang = cp.tile([P, ETILES, 72], F32)
            nc.sync.dma_start(out=ang[:], in_=ent[:].rearrange("t p k -> p t k"))
            ec = cp.tile([P, ETILES + WTILES, 24, 8], F16)
            fac = cp.tile([P, ETILES + WTILES, 6, 8], F16)
            nc.sync.dma_start(out=ec[:, ETILES:], in_=wcoef_d[:])
            nc.sync.dma_start(out=fac[:, ETILES:], in_=wfac_d[:])
            sidx = cp.tile([P, NT], I32)
            nc.sync.dma_start(out=sidx[:], in_=sidx_d[:])
            oidx = cp.tile([P, NT], I32)
            nc.sync.dma_start(out=oidx[:], in_=oidx_d[:])
            ident = cp.tile([P, P], F16)
            nc.sync.dma_start(out=ident[:], in_=ident_d[:])

            pm = cp.tile([P, 2], F16)
            nc.vector.memset(pm[:, 0:1], -1.0)
            nc.vector.memset(pm[:, 1:2], 1.0)

            cdb = cp.tile([P, 3], F32)
            nc.vector.memset(cdb[:, 0:1], 0.0)
            nc.vector.memset(cdb[:, 1:2], 0.5)
            nc.vector.memset(cdb[:, 2:3], PI / 2)
            nc.const_aps.aps[(F32, 0.0)] = cdb[:, 0:1]
            nc.const_aps.aps[(F32, 0.5)] = cdb[:, 1:2]
            nc.const_aps.aps[(F32, PI / 2)] = cdb[:, 2:3]

            # ---------------- entity coeffs (A part) ----------------
            av = ang[:].rearrange("p t (g a) -> p t g a", g=24, a=3)
            phi, tha, omg = av[:, :, :, 0], av[:, :, :, 1], av[:, :, :, 2]
            s1 = cp.tile([P, ETILES, 24], F32)
            s2 = cp.tile([P, ETILES, 24], F32)
            nc.vector.tensor_tensor(out=s1[:], in0=phi, in1=omg, op=ALU.add)
            nc.vector.tensor_tensor(out=s2[:], in0=phi, in1=omg, op=ALU.subtract)

            half = cp.tile([P, ETILES, 6, 24], F32)
            trig = cp.tile([P, ETILES, 6, 24], F32)
            hv, tv = half[:], trig[:]
            for i, srcv in ((0, tha), (2, s1[:]), (4, s2[:])):
                nc.vector.tensor_scalar(
                    out=hv[:, :, i], in0=srcv, scalar1=0.5, scalar2=PI / 2,
                    op0=ALU.mult, op1=ALU.add,
                )
                nc.vector.tensor_scalar_mul(hv[:, :, i + 1], srcv, 0.5)
            for i in range(6):
                nc.scalar.activation(out=tv[:, :, i], in_=hv[:, :, i], func=ACTFN.Sin)

            # products -> pair slots (v0,v0, -v1,v1, v2,v2, -v3,v3), fp16
            eA = ec[:, 0:ETILES]
            ch, sh = tv[:, :, 0], tv[:, :, 1]
            ca, sa = tv[:, :, 2], tv[:, :, 3]
            cb, sb = tv[:, :, 4], tv[:, :, 5]

            def u1(x):
                return x.rearrange("p t (g two) -> p t g two", two=1)

            def b2(x):
                return u1(x).to_broadcast([P, ETILES, 24, 2])

            TT = nc.vector.tensor_tensor
            TS = nc.vector.tensor_scalar_mul
            TT(out=eA[:, :, :, 0:2], in0=b2(ch), in1=b2(ca), op=ALU.mult)
            TT(out=eA[:, :, :, 3:4], in0=u1(ch), in1=u1(sa), op=ALU.mult)
            TS(eA[:, :, :, 2:3], eA[:, :, :, 3:4], -1.0)
            TT(out=eA[:, :, :, 4:6], in0=b2(sh), in1=b2(cb), op=ALU.mult)
            TT(out=eA[:, :, :, 7:8], in0=u1(sh), in1=u1(sb), op=ALU.mult)
            TS(eA[:, :, :, 6:7], eA[:, :, :, 7:8], -1.0)

            # layer-0 |+> doubling factors; step k expands wire 5-k (gate 5-k)
            fA = fac[:, 0:ETILES]
            rev = eA[:, :, 5::-1, :]
            v0r, v1r = rev[:, :, :, 0:1], rev[:, :, :, 3:4]
            v2r, v3r = rev[:, :, :, 4:5], rev[:, :, :, 7:8]
            tmp6 = cp.tile([P, ETILES, 6, 1], F32)
            t6 = tmp6[:]

            def fpair(dst_lo, n, scale):
                src = t6.to_broadcast([P, ETILES, 6, n]) if n == 2 else t6
                TS(fA[:, :, :, dst_lo : dst_lo + n], src, scale)

            TT(out=t6, in0=v0r, in1=v2r, op=ALU.subtract)
            fpair(0, 2, R2)                       # (f0r, f0r)
            TT(out=t6, in0=v1r, in1=v3r, op=ALU.add)
            fpair(2, 1, R2)                       # -f0i  (f0i = -(v1+v3)*r2)
            fpair(3, 1, -R2)                      # +f0i
            TT(out=t6, in0=v0r, in1=v2r, op=ALU.add)
            fpair(4, 2, R2)                       # (f1r, f1r)
            TT(out=t6, in0=v1r, in1=v3r, op=ALU.subtract)
            fpair(6, 1, -R2)                      # -f1i  (f1i = (v1-v3)*r2)
            fpair(7, 1, R2)                       # +f1i

            # ---------------- phase A evolution ----------------
            stA = cp.tile([P, ETILES, P], F16)
            SPL = 8  # tiles evolved on DVE; rest on Pool (idle during phase A)
            _emit_doubling(nc, gp, stA[:, 0:SPL], fac[:, 0:SPL], SPL, "adb")
            _emit_doubling(
                nc, gp, stA[:, SPL:ETILES], fac[:, SPL:ETILES],
                ETILES - SPL, "pdb", eng=nc.gpsimd,
            )
            for g, (c, t) in enumerate(CROTS):
                _emit_crot(
                    nc, gp, stA[:, 0:SPL], ec[:, 0:SPL], SPL, 6 + g, c, t, "ac"
                )
                _emit_crot(
                    nc, gp, stA[:, SPL:ETILES], ec[:, SPL:ETILES], ETILES - SPL,
                    6 + g, c, t, "pc", eng=nc.gpsimd,
                )

            # T store + AllGather
            nc.sync.dma_start(
                out=T_loc[:].rearrange("(t p) k -> p t k", p=P), in_=stA[:]
            )
            if no_collective:
                nc.sync.dma_start(out=T_full[0:EPC, :], in_=T_loc[:])
            else:
                nc.gpsimd.collective_compute(
                    "AllGather",
                    ALU.bypass,
                    ins=[T_loc[:]],
                    outs=[T_full[:]],
                    replica_groups=[list(range(NCORES))],
                )

            # ---------------- phase W evolution ----------------
            stW = cp.tile([P, WTILES, P], F16)
            _emit_doubling(nc, gp, stW[:], fac[:, ETILES:], WTILES, "wdb")
            for g, (c, t) in enumerate(CROTS):
                _emit_crot(nc, gp, stW[:], ec[:, ETILES:], WTILES, 6 + g, c, t, "wc")

            # W^T slots into SBUF: even rows = basis states, odd = swap-negate
            tempw = cp.tile([P, WTILES, P], F16)
            sw_in = stW[:].rearrange("p n (w two) -> p (n w) two", two=2)[:, :, ::-1]
            pmb = pm[:].rearrange("p (o two) -> p o two", o=1).to_broadcast(
                [P, WTILES * NA, 2]
            )
            nc.vector.tensor_tensor(
                out=tempw[:].rearrange("p n (w two) -> p (n w) two", two=2),
                in0=sw_in, in1=pmb, op=ALU.mult,
            )
            wl = W_loc[:].rearrange("wt h j two k -> (h j) wt two k")
            nc.sync.dma_start(out=wl[:, :, 0, :], in_=stW[:])
            nc.sync.dma_start(out=wl[:, :, 1, :], in_=tempw[:])
            wsb = cp.tile([P, RSLOT, P], F16)
            nc.sync.dma_start(
                out=wsb[:],
                in_=W_loc[:].rearrange("wt h j two k -> (j two) (wt h) k"),
            )

            # ---------------- phase C ----------------
            scores = cp.tile([P, NT], F32)
            T_flat = T_full[:]
            for t in range(NT):
                gts = gtsp.tile([P, P], F16, tag="gts")
                nc.gpsimd.indirect_dma_start(
                    out=gts[:], out_offset=None, in_=T_flat,
                    in_offset=bass.IndirectOffsetOnAxis(
                        ap=sidx[:, t : t + 1], axis=0),
                )
                gto = gtop.tile([P, P], F16, tag="gto")
                nc.gpsimd.indirect_dma_start(
                    out=gto[:], out_offset=None, in_=T_flat,
                    in_offset=bass.IndirectOffsetOnAxis(
                        ap=oidx[:, t : t + 1], axis=0),
                )
                psT = psY.tile([P, P], F32, tag="pst")
                nc.tensor.matmul(
                    out=psT[:], lhsT=gts[:], rhs=ident[:], start=True, stop=True
                )
                tst = tstp.tile([P, P], F16, tag="tst")
                nc.vector.tensor_copy(out=tst[:], in_=psT[:])
                pY = psY.tile([P, P], F32, tag="py")
                nc.tensor.matmul(
                    out=pY[:], lhsT=tst[:], rhs=wsb[:, t // TPS, :],
                    start=True, stop=True,
                )
                scr = scrp.tile([P, P], F32, tag="scr")
                nc.vector.tensor_tensor(
                    out=scr[:], in0=gto[:], in1=pY[:], op=ALU.mult
                )
                sdum = scrp.tile([P, P], F32, tag="sdum")
                nc.scalar.activation(
                    out=sdum[:], in_=scr[:], func=ACTFN.Copy,
                    accum_out=scores[:, t : t + 1],
                )

            nc.sync.dma_start(out=scores_d[:], in_=scores[:])

    nc.finalize()
    return nc


# --------------------------------------------------------------------------
# host side
# --------------------------------------------------------------------------


def _rot_elems(params):
    """params [..., 3] (phi, theta, omega) -> v0, v1, v2, v3 arrays.

    m00=(v0,-v1) m01=(-v2,-v3) m10=(v2,-v3) m11=(v0,v1)
    """
    phi, tha, omg = params[..., 0], params[..., 1], params[..., 2]
    ch, sh = np.cos(tha / 2), np.sin(tha / 2)
    a, b = (phi + omg) / 2, (phi - omg) / 2
    return ch * np.cos(a), ch * np.sin(a), sh * np.cos(b), sh * np.sin(b)


def _host_prep(entity_params, relation_params, s_idx, p_idx, o_idx):
    ent = np.asarray(entity_params, dtype=np.float32)
    rel = np.asarray(relation_params, dtype=np.float32)
    s_idx = np.asarray(s_idx)
    p_idx = np.asarray(p_idx)
    o_idx = np.asarray(o_idx)

    # ---- entity shards ----
    ent_flat = ent.reshape(E, 72)
    ent_pad = np.zeros((EPAD, 72), np.float32)
    ent_pad[:E] = ent_flat
    ent_shards = [
        ent_pad[c * EPC : (c + 1) * EPC].reshape(ETILES, P, 72) for c in range(NCORES)
    ]

    # ---- p-sorted stream, contiguous core slices ----
    order = np.argsort(p_idx, kind="stable")
    per = B // NCORES
    gate_rel = rel.reshape(R, 24, 3)
    v0, v1, v2, v3 = _rot_elems(gate_rel)  # each [R, 24]

    in_maps = []
    outpos = np.full((NCORES, NT, P), -1, np.int64)
    for c in range(NCORES):
        sl = order[c * per : (c + 1) * per]
        rels_c = p_idx[sl]
        rels, starts = np.unique(rels_c, return_index=True)
        assert len(rels) <= RSLOT, f"core {c} has {len(rels)} relations"

        sidx = np.zeros((P, NT), np.int32)
        oidx = np.zeros((P, NT), np.int32)
        for s, r in enumerate(rels):
            elems = sl[rels_c == r]
            assert len(elems) <= TPS * P, f"relation {r} needs >3 tiles"
            for k in range(TPS):
                seg = elems[k * P : (k + 1) * P]
                n = len(seg)
                if n == 0:
                    break
                t = s * TPS + k
                sidx[:n, t] = s_idx[seg]
                oidx[:n, t] = o_idx[seg]
                outpos[c, t, :n] = seg

        # W coeff pair table [P, WTILES, 24, 8]
        wcoef = np.zeros((P, WTILES, 24, 8), np.float16)
        wfac = np.zeros((P, WTILES, 6, 8), np.float16)
        j = np.arange(NA)
        for s, r in enumerate(rels):
            wt, hhalf = divmod(s, 2)
            rows = slice(NA * hhalf, NA * hhalf + NA)
            for g in range(24):
                vals = (v0[r, g], v0[r, g], -v1[r, g], v1[r, g],
                        v2[r, g], v2[r, g], -v3[r, g], v3[r, g])
                for kk, vv in enumerate(vals):
                    wcoef[rows, wt, g, kk] = vv
            # basis doubling factors: step k expands wire 5-k; basis j bit
            # of wire q=5-k is (j >> k) & 1
            for k in range(6):
                qg = 5 - k
                bit = (j >> k) & 1
                m00 = (v0[r, qg], -v1[r, qg])
                m01 = (-v2[r, qg], -v3[r, qg])
                m10 = (v2[r, qg], -v3[r, qg])
                m11 = (v0[r, qg], v1[r, qg])
                wfac[rows, wt, k, 0] = np.where(bit == 0, m00[0], m01[0])
                wfac[rows, wt, k, 1] = wfac[rows, wt, k, 0]
                u0i = np.where(bit == 0, m00[1], m01[1])
                wfac[rows, wt, k, 2] = -u0i
                wfac[rows, wt, k, 3] = u0i
                wfac[rows, wt, k, 4] = np.where(bit == 0, m10[0], m11[0])
                wfac[rows, wt, k, 5] = wfac[rows, wt, k, 4]
                u1i = np.where(bit == 0, m10[1], m11[1])
                wfac[rows, wt, k, 6] = -u1i
                wfac[rows, wt, k, 7] = u1i

        in_maps.append(
            {
                "ent_par": ent_shards[c],
                "wcoef": wcoef,
                "wfac": wfac,
                "sidx": sidx,
                "oidx": oidx,
                "ident": np.eye(P, dtype=np.float16),
            }
        )
    return in_maps, outpos


_PROGRAM = None


def kernel(entity_params, relation_params, s_idx, p_idx, o_idx):
    global _PROGRAM
    in_maps, outpos = _host_prep(entity_params, relation_params, s_idx, p_idx, o_idx)
    if _PROGRAM is None:
        _PROGRAM = build_program()
    nc = _PROGRAM
    res = run_bass_kernel_spmd(nc, in_maps, list(range(NCORES)))
    out = np.zeros(B, np.float32)
    for c in range(NCORES):
        sc = res.results[c]["scores"]  # [P, NT]
        pos = outpos[c]  # [NT, P]
        mask = pos >= 0
        out[pos[mask]] = sc.T[mask]
    return out


if __name__ == "__main__":
    build_program()
    print("build OK")


# revision 30
# speedup vs baseline: 1.9305x; 1.0460x over previous
"""Trainium2 Bass kernel for the quantum-circuit KG-embedding scoring model.

Math: score(s,p,o) = Re(<B_o h | W_p | B_s h>) where B_e / W_p are the
24-gate circuit blocks for entity/relation params and h = |+>^6.

v4 design (fp16, interleaved re/im pairs):
  State layout [128 part, nt, 128] fp16 where the 128 free elems are 64
  amplitudes as interleaved (re, im) pairs.  A 2x2 gate update is 14
  tensor_tensor ops (8 pair-coeff products + 6 adds; re/im swaps are
  negative-stride reads of contiguous temps), all DVE "2x_1p" eligible.

  Phase A: evolve 1280 local entities (10 tiles); store fp16 rows,
    AllGather -> T_full [10240, 128].
  Phase W: evolve 16 W tiles (2 relations x 64 basis cols each) on DVE
    WHILE phase C's gathers run on the Pool engine; W^T slots assembled
    into SBUF wsb [128, 32, 128] via 4 partition-strided SBUF->SBUF DMAs
    (even rows = basis states, odd rows = swap-negate).
  Phase C: host packs the p-sorted batch into a STATIC slot->tile map
    (slot s = t//3; every relation on a core needs <= 3 tiles, <= 32
    relations per core - verified for B=65536, R=200).  Per tile: two
    single-index indirect row-gathers (s, o), XBAR DMA transpose of Ts,
    PE matmul Y^T = Ts @ W^T_slot (static SBUF rhs), DVE product with
    To, ACT-engine accumulate into scores.

Host does only: trig for the 200-relation coeff tables (tiny), index
sort/packing, and output unpermute.
"""

import sys
import numpy as np

for _p in ("/opt/trn_rl_repo",):
    if _p not in sys.path:
        sys.path.insert(0, _p)

import concourse.bass as bass
import concourse.bacc as bacc
import concourse.mybir as mybir
from concourse import tile
from concourse.bass_utils import run_bass_kernel_spmd

F32 = mybir.dt.float32
F16 = mybir.dt.float16
I32 = mybir.dt.int32
ALU = mybir.AluOpType
ACTFN = mybir.ActivationFunctionType

P = 128
Q = 6
NA = 64                      # 2^Q amplitudes
NCORES = 8
E, R, B = 10000, 200, 65536
ETILES = 10                  # entity tiles per core
EPC = ETILES * P             # 1280 entities per core
EPAD = EPC * NCORES          # 10240 padded entity rows
WTILES = 14                  # W-phase tiles per core (2 relations each)
RSLOT = 2 * WTILES           # 28 relation slots per core
TPS = 3                      # static tiles per slot
NT = RSLOT * TPS             # 96 phase-C tiles per core
R2 = float(2.0 ** -0.5)
PI = float(np.pi)

# CRot gate list: (control, target) wire pairs, in circuit order
CROTS = [(q, (q + off) % Q) for off in (1, 2, 3) for q in range(Q)]


# --------------------------------------------------------------------------
# device program
# --------------------------------------------------------------------------


def _pair_bc(coef_ap, nt, nrep):
    """[p, nt, 2] pair slice -> broadcast to [p, nt, nrep, 2] (4-D)."""
    v = coef_ap.rearrange("p n (m two) -> p n m two", m=1, two=2)
    return v.to_broadcast([P, nt, nrep, 2])


def _emit_crot(nc, pool, st, ec, nt, g, c, t, tag, eng=None):
    """Apply CRot gate g (control c, target t) in place on st [P, nt, 128].

    ec: [P, nt, 24, 8] fp16 pair-coeff slots (v0,v0, -v1,v1, v2,v2, -v3,v3).
    """
    cpos, tpos = 5 - c, 5 - t
    hi, lo = max(cpos, tpos), min(cpos, tpos)
    A = 1 << (5 - hi)
    Bm = 1 << (hi - lo - 1)
    C = 1 << lo
    v = st.rearrange(
        "p n (a x b y c two) -> p n a x b y c two",
        a=A, x=2, b=Bm, y=2, c=C, two=2,
    )
    cbit_is_x = cpos == hi

    def sel(cv, tv):
        xv, yv = (cv, tv) if cbit_is_x else (tv, cv)
        return v[:, :, :, xv, :, yv, :, :]

    a0 = sel(1, 0)   # [p, n, A, Bm, C, 2]
    a1 = sel(1, 1)
    L = A * Bm * C * 2  # 32

    def co(lo_s):
        return _pair_bc(ec[:, :, g, lo_s : lo_s + 2], nt, A * Bm * C)

    def qt(tg):
        tt = pool.tile([P, nt, L], F16, tag=tag + tg)
        return tt

    def as5(tt):
        return tt[:].rearrange(
            "p n (a b c two) -> p n a b c two", a=A, b=Bm, c=C, two=2
        )

    def flat(tt):
        return tt[:].rearrange("p n (l two) -> p n l two", two=2)

    def swap(tt):
        return flat(tt)[:, :, :, ::-1]

    c0, c1, c2, c3 = co(0), co(2), co(4), co(6)
    q1, q2, q3, q4 = qt("q1"), qt("q2"), qt("q3"), qt("q4")
    q5, q6, q7, q8 = qt("q5"), qt("q6"), qt("q7"), qt("q8")
    TT = (eng or nc.vector).tensor_tensor
    TT(out=as5(q1), in0=a0, in1=c0, op=ALU.mult)
    TT(out=as5(q2), in0=a0, in1=c1, op=ALU.mult)
    TT(out=as5(q3), in0=a1, in1=c2, op=ALU.mult)
    TT(out=as5(q4), in0=a1, in1=c3, op=ALU.mult)
    TT(out=as5(q5), in0=a0, in1=c2, op=ALU.mult)
    TT(out=as5(q6), in0=a0, in1=c3, op=ALU.mult)
    TT(out=as5(q7), in0=a1, in1=c0, op=ALU.mult)
    TT(out=as5(q8), in0=a1, in1=c1, op=ALU.mult)
    pa, pb = qt("pa"), qt("pb")
    pc, pd = qt("pc"), qt("pd")
    TT(out=flat(pa), in0=flat(q1), in1=swap(q2), op=ALU.add)
    TT(out=flat(pb), in0=flat(q3), in1=swap(q4), op=ALU.subtract)
    TT(out=flat(pc), in0=flat(q5), in1=swap(q6), op=ALU.add)
    TT(out=flat(pd), in0=flat(q7), in1=swap(q8), op=ALU.subtract)
    TT(out=a0, in0=as5(pa), in1=as5(pb), op=ALU.subtract)
    TT(out=a1, in0=as5(pc), in1=as5(pd), op=ALU.add)


def _pair_bc_db(coef_ap, nt, w):
    v = coef_ap.rearrange("p n (w two) -> p n w two", w=1, two=2)
    return v.to_broadcast([P, nt, w, 2])


def _emit_doubling(nc, pool, st, fac, nt, tag, eng=None):
    """Product-state doubling, in place on st [P, nt, 128] fp16.

    fac [P, nt, 6, 8] fp16 slots per step: (f0r,f0r, -f0i,f0i, f1r,f1r,
    -f1i,f1i); step k expands amplitude bit k (wire 5-k).
    """
    CP = (eng or nc.vector).tensor_copy
    TT = (eng or nc.vector).tensor_tensor
    # seed from step-0 factors: amp0 = f0, amp1 = f1
    CP(out=st[:, :, 0:1], in_=fac[:, :, 0, 0:1])
    CP(out=st[:, :, 1:2], in_=fac[:, :, 0, 3:4])
    CP(out=st[:, :, 2:3], in_=fac[:, :, 0, 4:5])
    CP(out=st[:, :, 3:4], in_=fac[:, :, 0, 7:8])
    for k in range(1, 6):
        w = 1 << k  # current state width in pairs
        cview = st[:, :, 0 : 2 * w].rearrange("p n (w two) -> p n w two", two=2)
        for m in (1, 0):  # m=1 writes fresh upper half first
            frp = _pair_bc_db(fac[:, :, k, 4 * m : 4 * m + 2], nt, w)
            fim = _pair_bc_db(fac[:, :, k, 4 * m + 2 : 4 * m + 4], nt, w)
            t1 = pool.tile([P, nt, 2 * w], F16, tag=tag + "A")
            t2 = pool.tile([P, nt, 2 * w], F16, tag=tag + "B")
            t1v = t1[:].rearrange("p n (w two) -> p n w two", two=2)
            t2v = t2[:].rearrange("p n (w two) -> p n w two", two=2)
            TT(out=t1v, in0=cview, in1=frp, op=ALU.mult)
            TT(out=t2v, in0=cview, in1=fim, op=ALU.mult)
            t1f = t1[:].rearrange("p n (w two) -> p n w two", two=2)
            t2s = t2[:].rearrange("p n (w two) -> p n w two", two=2)[:, :, :, ::-1]
            df = st[:, :, m * 2 * w : (m + 1) * 2 * w].rearrange(
                "p n (w two) -> p n w two", two=2
            )
            TT(out=df, in0=t1f, in1=t2s, op=ALU.subtract)


def build_program(no_collective=False):
    nc = bacc.Bacc("TRN2", target_bir_lowering=False, debug=False)

    ent = nc.dram_tensor("ent_par", [ETILES, P, 72], F32, kind="ExternalInput")
    wcoef_d = nc.dram_tensor("wcoef", [P, WTILES, 24, 8], F16, kind="ExternalInput")
    wfac_d = nc.dram_tensor("wfac", [P, WTILES, 6, 8], F16, kind="ExternalInput")
    sidx_d = nc.dram_tensor("sidx", [P, NT], I32, kind="ExternalInput")
    oidx_d = nc.dram_tensor("oidx", [P, NT], I32, kind="ExternalInput")
    ident_d = nc.dram_tensor("ident", [P, P], F16, kind="ExternalInput")
    scores_d = nc.dram_tensor("scores", [P, NT], F32, kind="ExternalOutput")

    with tile.TileContext(nc) as tc:
        with (
            tc.tile_pool(name="const", bufs=1) as cp,
            tc.tile_pool(name="gtmp", bufs=2) as gp,
            tc.tile_pool(name="gts", bufs=12) as gtsp,
            tc.tile_pool(name="gto", bufs=12) as gtop,
            tc.tile_pool(name="tst", bufs=8) as tstp,
            tc.tile_pool(name="scr", bufs=6) as scrp,
            tc.tile_pool(name="cpy", bufs=4, space="PSUM") as psY,
            tc.tile_pool(name="dram", bufs=1, space="DRAM") as dp,
        ):
            # ---------------- DRAM scratch ----------------
            T_loc = dp.tile([EPC, P], F16)
            T_full = dp.tile([EPAD, P], F16, addr_space="Shared")
            W_loc = dp.tile([WTILES, 2, NA, 2, P], F16)

            # ---------------- load inputs ----------------
            ang = cp.tile([P, ETILES, 72], F32)
            nc.sync.dma_start(out=ang[:], in_=ent[:].rearrange("t p k -> p t k"))
            ec = cp.tile([P, ETILES + WTILES, 24, 8], F16)
            fac = cp.tile([P, ETILES + WTILES, 6, 8], F16)
            nc.sync.dma_start(out=ec[:, ETILES:], in_=wcoef_d[:])
            nc.sync.dma_start(out=fac[:, ETILES:], in_=wfac_d[:])
            sidx = cp.tile([P, NT], I32)
            nc.sync.dma_start(out=sidx[:], in_=sidx_d[:])
            oidx = cp.tile([P, NT], I32)
            nc.sync.dma_start(out=oidx[:], in_=oidx_d[:])
            ident = cp.tile([P, P], F16)
            nc.sync.dma_start(out=ident[:], in_=ident_d[:])

            pm = cp.tile([P, 2], F16)
            nc.vector.memset(pm[:, 0:1], -1.0)
            nc.vector.memset(pm[:, 1:2], 1.0)

            cdb = cp.tile([P, 3], F32)
            nc.vector.memset(cdb[:, 0:1], 0.0)
            nc.vector.memset(cdb[:, 1:2], 0.5)
            nc.vector.memset(cdb[:, 2:3], PI / 2)
            nc.const_aps.aps[(F32, 0.0)] = cdb[:, 0:1]
            nc.const_aps.aps[(F32, 0.5)] = cdb[:, 1:2]
            nc.const_aps.aps[(F32, PI / 2)] = cdb[:, 2:3]

            # ---------------- entity coeffs (A part) ----------------
            av = ang[:].rearrange("p t (g a) -> p t g a", g=24, a=3)
            phi, tha, omg = av[:, :, :, 0], av[:, :, :, 1], av[:, :, :, 2]
            s1 = cp.tile([P, ETILES, 24], F32)
            s2 = cp.tile([P, ETILES, 24], F32)
            nc.vector.tensor_tensor(out=s1[:], in0=phi, in1=omg, op=ALU.add)
            nc.vector.tensor_tensor(out=s2[:], in0=phi, in1=omg, op=ALU.subtract)

            half = cp.tile([P, ETILES, 6, 24], F32)
            trig = cp.tile([P, ETILES, 6, 24], F32)
            hv, tv = half[:], trig[:]
            for i, srcv in ((0, tha), (2, s1[:]), (4, s2[:])):
                nc.vector.tensor_scalar(
                    out=hv[:, :, i], in0=srcv, scalar1=0.5, scalar2=PI / 2,
                    op0=ALU.mult, op1=ALU.add,
                )
                nc.vector.tensor_scalar_mul(hv[:, :, i + 1], srcv, 0.5)
            for i in range(6):
                nc.scalar.activation(out=tv[:, :, i], in_=hv[:, :, i], func=ACTFN.Sin)

            # products -> pair slots (v0,v0, -v1,v1, v2,v2, -v3,v3), fp16
            eA = ec[:, 0:ETILES]
            ch, sh = tv[:, :, 0], tv[:, :, 1]
            ca, sa = tv[:, :, 2], tv[:, :, 3]
            cb, sb = tv[:, :, 4], tv[:, :, 5]

            def u1(x):
                return x.rearrange("p t (g two) -> p t g two", two=1)

            def b2(x):
                return u1(x).to_broadcast([P, ETILES, 24, 2])

            TT = nc.vector.tensor_tensor
            TS = nc.vector.tensor_scalar_mul
            TT(out=eA[:, :, :, 0:2], in0=b2(ch), in1=b2(ca), op=ALU.mult)
            TT(out=eA[:, :, :, 3:4], in0=u1(ch), in1=u1(sa), op=ALU.mult)
            TS(eA[:, :, :, 2:3], eA[:, :, :, 3:4], -1.0)
            TT(out=eA[:, :, :, 4:6], in0=b2(sh), in1=b2(cb), op=ALU.mult)
            TT(out=eA[:, :, :, 7:8], in0=u1(sh), in1=u1(sb), op=ALU.mult)
            TS(eA[:, :, :, 6:7], eA[:, :, :, 7:8], -1.0)

            # layer-0 |+> doubling factors; step k expands wire 5-k (gate 5-k)
            fA = fac[:, 0:ETILES]
            rev = eA[:, :, 5::-1, :]
            v0r, v1r = rev[:, :, :, 0:1], rev[:, :, :, 3:4]
            v2r, v3r = rev[:, :, :, 4:5], rev[:, :, :, 7:8]
            tmp6 = cp.tile([P, ETILES, 6, 1], F32)
            t6 = tmp6[:]

            def fpair(dst_lo, n, scale):
                src = t6.to_broadcast([P, ETILES, 6, n]) if n == 2 else t6
                TS(fA[:, :, :, dst_lo : dst_lo + n], src, scale)

            TT(out=t6, in0=v0r, in1=v2r, op=ALU.subtract)
            fpair(0, 2, R2)                       # (f0r, f0r)
            TT(out=t6, in0=v1r, in1=v3r, op=ALU.add)
            fpair(2, 1, R2)                       # -f0i  (f0i = -(v1+v3)*r2)
            fpair(3, 1, -R2)                      # +f0i
            TT(out=t6, in0=v0r, in1=v2r, op=ALU.add)
            fpair(4, 2, R2)                       # (f1r, f1r)
            TT(out=t6, in0=v1r, in1=v3r, op=ALU.subtract)
            fpair(6, 1, -R2)                      # -f1i  (f1i = (v1-v3)*r2)
            fpair(7, 1, R2)                       # +f1i

            # ---------------- phase A evolution ----------------
            stA = cp.tile([P, ETILES, P], F16)
            SPL = 8  # tiles evolved on DVE; rest on Pool (idle during phase A)
            _emit_doubling(nc, gp, stA[:, 0:SPL], fac[:, 0:SPL], SPL, "adb")
            _emit_doubling(
                nc, gp, stA[:, SPL:ETILES], fac[:, SPL:ETILES],
                ETILES - SPL, "pdb", eng=nc.gpsimd,
            )
            for g, (c, t) in enumerate(CROTS):
                _emit_crot(
                    nc, gp, stA[:, 0:SPL], ec[:, 0:SPL], SPL, 6 + g, c, t, "ac"
                )
                _emit_crot(
                    nc, gp, stA[:, SPL:ETILES], ec[:, SPL:ETILES], ETILES - SPL,
                    6 + g, c, t, "pc", eng=nc.gpsimd,
                )

            # T store (AllGather emitted after W-evolution so the DVE
            # instruction stream is never blocked behind collective-dependent
            # consumers)
            nc.sync.dma_start(
                out=T_loc[:].rearrange("(t p) k -> p t k", p=P), in_=stA[:]
            )

            # ---------------- phase W evolution ----------------
            stW = cp.tile([P, WTILES, P], F16)
            _emit_doubling(nc, gp, stW[:], fac[:, ETILES:], WTILES, "wdb")
            for g, (c, t) in enumerate(CROTS):
                _emit_crot(nc, gp, stW[:], ec[:, ETILES:], WTILES, 6 + g, c, t, "wc")

            # W^T slots into SBUF: even rows = basis states, odd = swap-negate
            tempw = cp.tile([P, WTILES, P], F16)
            sw_in = stW[:].rearrange("p n (w two) -> p (n w) two", two=2)[:, :, ::-1]
            pmb = pm[:].rearrange("p (o two) -> p o two", o=1).to_broadcast(
                [P, WTILES * NA, 2]
            )
            nc.vector.tensor_tensor(
                out=tempw[:].rearrange("p n (w two) -> p (n w) two", two=2),
                in0=sw_in, in1=pmb, op=ALU.mult,
            )
            wl = W_loc[:].rearrange("wt h j two k -> (h j) wt two k")
            nc.sync.dma_start(out=wl[:, :, 0, :], in_=stW[:])
            nc.sync.dma_start(out=wl[:, :, 1, :], in_=tempw[:])
            wsb = cp.tile([P, RSLOT, P], F16)
            nc.sync.dma_start(
                out=wsb[:],
                in_=W_loc[:].rearrange("wt h j two k -> (j two) (wt h) k"),
            )

            if no_collective:
                nc.sync.dma_start(out=T_full[0:EPC, :], in_=T_loc[:])
            else:
                nc.gpsimd.collective_compute(
                    "AllGather",
                    ALU.bypass,
                    ins=[T_loc[:]],
                    outs=[T_full[:]],
                    replica_groups=[list(range(NCORES))],
                )

            # ---------------- phase C ----------------
            scores = cp.tile([P, NT], F32)
            T_flat = T_full[:]
            for t in range(NT):
                gts = gtsp.tile([P, P], F16, tag="gts")
                nc.gpsimd.indirect_dma_start(
                    out=gts[:], out_offset=None, in_=T_flat,
                    in_offset=bass.IndirectOffsetOnAxis(
                        ap=sidx[:, t : t + 1], axis=0),
                )
                gto = gtop.tile([P, P], F16, tag="gto")
                nc.gpsimd.indirect_dma_start(
                    out=gto[:], out_offset=None, in_=T_flat,
                    in_offset=bass.IndirectOffsetOnAxis(
                        ap=oidx[:, t : t + 1], axis=0),
                )
                psT = psY.tile([P, P], F32, tag="pst")
                nc.tensor.matmul(
                    out=psT[:], lhsT=gts[:], rhs=ident[:], start=True, stop=True
                )
                tst = tstp.tile([P, P], F16, tag="tst")
                nc.vector.tensor_copy(out=tst[:], in_=psT[:])
                pY = psY.tile([P, P], F32, tag="py")
                nc.tensor.matmul(
                    out=pY[:], lhsT=tst[:], rhs=wsb[:, t // TPS, :],
                    start=True, stop=True,
                )
                scr = scrp.tile([P, P], F32, tag="scr")
                nc.vector.tensor_tensor(
                    out=scr[:], in0=gto[:], in1=pY[:], op=ALU.mult
                )
                sdum = scrp.tile([P, P], F32, tag="sdum")
                nc.scalar.activation(
                    out=sdum[:], in_=scr[:], func=ACTFN.Copy,
                    accum_out=scores[:, t : t + 1],
                )

            nc.sync.dma_start(out=scores_d[:], in_=scores[:])

    nc.finalize()
    return nc


# --------------------------------------------------------------------------
# host side
# --------------------------------------------------------------------------


def _rot_elems(params):
    """params [..., 3] (phi, theta, omega) -> v0, v1, v2, v3 arrays.

    m00=(v0,-v1) m01=(-v2,-v3) m10=(v2,-v3) m11=(v0,v1)
    """
    phi, tha, omg = params[..., 0], params[..., 1], params[..., 2]
    ch, sh = np.cos(tha / 2), np.sin(tha / 2)
    a, b = (phi + omg) / 2, (phi - omg) / 2
    return ch * np.cos(a), ch * np.sin(a), sh * np.cos(b), sh * np.sin(b)


def _host_prep(entity_params, relation_params, s_idx, p_idx, o_idx):
    ent = np.asarray(entity_params, dtype=np.float32)
    rel = np.asarray(relation_params, dtype=np.float32)
    s_idx = np.asarray(s_idx)
    p_idx = np.asarray(p_idx)
    o_idx = np.asarray(o_idx)

    # ---- entity shards ----
    ent_flat = ent.reshape(E, 72)
    ent_pad = np.zeros((EPAD, 72), np.float32)
    ent_pad[:E] = ent_flat
    ent_shards = [
        ent_pad[c * EPC : (c + 1) * EPC].reshape(ETILES, P, 72) for c in range(NCORES)
    ]

    # ---- p-sorted stream, contiguous core slices ----
    order = np.argsort(p_idx, kind="stable")
    per = B // NCORES
    gate_rel = rel.reshape(R, 24, 3)
    v0, v1, v2, v3 = _rot_elems(gate_rel)  # each [R, 24]

    in_maps = []
    outpos = np.full((NCORES, NT, P), -1, np.int64)
    for c in range(NCORES):
        sl = order[c * per : (c + 1) * per]
        rels_c = p_idx[sl]
        rels, starts = np.unique(rels_c, return_index=True)
        assert len(rels) <= RSLOT, f"core {c} has {len(rels)} relations"

        sidx = np.zeros((P, NT), np.int32)
        oidx = np.zeros((P, NT), np.int32)
        for s, r in enumerate(rels):
            elems = sl[rels_c == r]
            assert len(elems) <= TPS * P, f"relation {r} needs >3 tiles"
            for k in range(TPS):
                seg = elems[k * P : (k + 1) * P]
                n = len(seg)
                if n == 0:
                    break
                t = s * TPS + k
                sidx[:n, t] = s_idx[seg]
                oidx[:n, t] = o_idx[seg]
                outpos[c, t, :n] = seg

        # W coeff pair table [P, WTILES, 24, 8]
        wcoef = np.zeros((P, WTILES, 24, 8), np.float16)
        wfac = np.zeros((P, WTILES, 6, 8), np.float16)
        j = np.arange(NA)
        for s, r in enumerate(rels):
            wt, hhalf = divmod(s, 2)
            rows = slice(NA * hhalf, NA * hhalf + NA)
            for g in range(24):
                vals = (v0[r, g], v0[r, g], -v1[r, g], v1[r, g],
                        v2[r, g], v2[r, g], -v3[r, g], v3[r, g])
                for kk, vv in enumerate(vals):
                    wcoef[rows, wt, g, kk] = vv
            # basis doubling factors: step k expands wire 5-k; basis j bit
            # of wire q=5-k is (j >> k) & 1
            for k in range(6):
                qg = 5 - k
                bit = (j >> k) & 1
                m00 = (v0[r, qg], -v1[r, qg])
                m01 = (-v2[r, qg], -v3[r, qg])
                m10 = (v2[r, qg], -v3[r, qg])
                m11 = (v0[r, qg], v1[r, qg])
                wfac[rows, wt, k, 0] = np.where(bit == 0, m00[0], m01[0])
                wfac[rows, wt, k, 1] = wfac[rows, wt, k, 0]
                u0i = np.where(bit == 0, m00[1], m01[1])
                wfac[rows, wt, k, 2] = -u0i
                wfac[rows, wt, k, 3] = u0i
                wfac[rows, wt, k, 4] = np.where(bit == 0, m10[0], m11[0])
                wfac[rows, wt, k, 5] = wfac[rows, wt, k, 4]
                u1i = np.where(bit == 0, m10[1], m11[1])
                wfac[rows, wt, k, 6] = -u1i
                wfac[rows, wt, k, 7] = u1i

        in_maps.append(
            {
                "ent_par": ent_shards[c],
                "wcoef": wcoef,
                "wfac": wfac,
                "sidx": sidx,
                "oidx": oidx,
                "ident": np.eye(P, dtype=np.float16),
            }
        )
    return in_maps, outpos


_PROGRAM = None


def kernel(entity_params, relation_params, s_idx, p_idx, o_idx):
    global _PROGRAM
    in_maps, outpos = _host_prep(entity_params, relation_params, s_idx, p_idx, o_idx)
    if _PROGRAM is None:
        _PROGRAM = build_program()
    nc = _PROGRAM
    res = run_bass_kernel_spmd(nc, in_maps, list(range(NCORES)))
    out = np.zeros(B, np.float32)
    for c in range(NCORES):
        sc = res.results[c]["scores"]  # [P, NT]
        pos = outpos[c]  # [NT, P]
        mask = pos >= 0
        out[pos[mask]] = sc.T[mask]
    return out


if __name__ == "__main__":
    build_program()
    print("build OK")
